# revision 10
# baseline (speedup 1.0000x reference)
"""Trainium2 Bass kernel for nn_BertAoA_Decoder_Core (6-layer BERT-style encoder,
layer-0 cross-attention to p_att_feats).

Strategy: pure data-parallel over batch across 8 NeuronCores (8 examples/core).
v2: fp8e4m3 DoubleRow matmuls (2x PE rate) for the Q/K/V/O projections with
power-of-2 weight/activation scaling (descale folded into PSUM evictions);
bf16 FFN weights/activations; fp32 residual stream; global tile pools and an
emission order that keeps the TensorE instruction stream dense so the PE
p-state stays at full clock.
"""

import sys

sys.path.insert(0, "/opt/trn_rl_repo")

import numpy as np
import ml_dtypes
from contextlib import ExitStack

import concourse.bass as bass
import concourse.mybir as mybir
import concourse.tile as tile
from concourse.masks import make_identity
from concourse.bass_utils import run_bass_kernel_spmd

F32 = mybir.dt.float32
BF16 = mybir.dt.bfloat16
FP8 = mybir.dt.float8e4
AX = mybir.AxisListType.X
OP = mybir.AluOpType
ACT = mybir.ActivationFunctionType
DR = mybir.MatmulPerfMode.DoubleRow

NP_BF16 = ml_dtypes.bfloat16
NP_FP8 = ml_dtypes.float8_e4m3

# Problem constants (hardcoded per contract)
B, S, C, D, H, L, F = 64, 128, 196, 1024, 16, 6, 4096
DK = D // H              # 64
NCORES = 8
BL = B // NCORES         # 8 examples per core
T = BL * S               # 1024 query tokens per core
KD = D // 128            # 8 contraction tiles
KDH = KD // 2            # 4 contraction pair-tiles (DoubleRow)
MD = D // 128            # 8 output tiles
FT = F // 128            # 32 FFN f-tiles
NFB = 8                  # FFN f-blocks
FBT = FT // NFB          # 4 f-tiles per block
NGRP = 2                 # example groups per core (== token halves)
GE = BL // NGRP          # 4 examples per group
GT = GE * S              # 512 tokens per group
GC0 = GE * C             # 784 context tokens per group (layer 0)
LN_EPS = 1e-6
HSC = 16.0               # fp8 activation scale


def _split_multi_waits(nc):
    """This container's walrus accepts only one sync-wait per CTRL instruction;
    hoist extra waits onto preceding NoOps on the same engine."""
    cnt = 0
    for fn in nc.m.functions:
        for bb in fn.blocks:
            new_list = []
            for ins in bb.instructions:
                si = getattr(ins, "sync_info", None)
                ow = getattr(si, "on_wait", None) if si is not None else None
                if ow and len(ow) > 1:
                    for w in ow[:-1]:
                        nop = mybir.InstNoOp(
                            name=f"{ins.name}-wsplit-{cnt}",
                            engine=ins.engine,
                            sync_info=mybir.SyncInfo(on_wait=[w], on_update=[]),
                        )
                        cnt += 1
                        new_list.append(nop)
                    si.on_wait = [ow[-1]]
                new_list.append(ins)
            bb.instructions = new_list
    return cnt


def _newton_rsqrt(nc, pool, v_ap, out_ap, n):
    """out = 1/sqrt(v) elementwise on a small [128, n] fp32 AP, DVE-only.

    y0 = 0.5*(1 + 1/v), then 4 Newton iterations y <- y*(1.5 - 0.5*v*y^2)."""
    r = pool.tile([128, n], F32, name="rs_r", tag="rs_r")
    t = pool.tile([128, n], F32, name="rs_t", tag="rs_t")
    nc.vector.reciprocal(r, v_ap)
    nc.vector.tensor_scalar(out_ap, r, 0.5, 0.5, OP.mult, OP.add)
    for _ in range(4):
        nc.vector.tensor_tensor(t, out_ap, out_ap, OP.mult)
        nc.vector.tensor_tensor(t, t, v_ap, OP.mult)
        nc.vector.tensor_scalar(t, t, -0.5, 1.5, OP.mult, OP.add)
        nc.vector.tensor_tensor(out_ap, out_ap, t, OP.mult)


def _layer_norm_half(nc, stats_pool, x_tiles, h_pool, tagpfx, htag):
    """Pre-norm (x-mu)*rstd for 4 [128, D] fp32 token-major tiles -> bf16."""
    nb = len(x_tiles)
    stat = stats_pool.tile([128, nb, 12], F32, name=f"{tagpfx}_stat",
                           tag="ln_stat")
    mv = stats_pool.tile([128, nb, 2], F32, name=f"{tagpfx}_mv", tag="ln_mv")
    var = stats_pool.tile([128, nb], F32, name=f"{tagpfx}_var", tag="ln_var")
    rst = stats_pool.tile([128, nb], F32, name=f"{tagpfx}_rst", tag="ln_rst")
    for i in range(nb):
        nc.vector.bn_stats(stat[:, i, 0:6], x_tiles[i][:, 0:512])
        nc.vector.bn_stats(stat[:, i, 6:12], x_tiles[i][:, 512:1024])
        nc.vector.bn_aggr(mv[:, i, :], stat[:, i, :])
    nc.vector.tensor_scalar(var, mv[:, :, 1], LN_EPS, None, OP.add)
    _newton_rsqrt(nc, stats_pool, var, rst, nb)
    h_tiles = []
    for i in range(nb):
        h = h_pool.tile([128, D], BF16, name=f"{tagpfx}_h{i}", tag=htag)
        nc.vector.tensor_scalar(h, x_tiles[i], mv[:, i, 0:1],
                                rst[:, i : i + 1], OP.subtract, OP.mult)
        h_tiles.append(h)
    return h_tiles


def build_program(meta, n_layers=L):
    """meta: dict from prepare_host with descale constants and bias flags."""
    SQ, SK, SV, SO = meta["SQ"], meta["SK"], meta["SV"], meta["SO"]
    nonzero_bq, nonzero_b1 = meta["nonzero_bq"], meta["nonzero_b1"]
    nonzero_bo, nonzero_b2 = meta["nonzero_bo"], meta["nonzero_b2"]

    nc = bass.Bass()
    x_in = nc.declare_dram_parameter("x", [T, D], F32, isOutput=False)
    y_out = nc.declare_dram_parameter("y", [T, D], F32, isOutput=True)
    kv0_d = nc.declare_dram_parameter("kv0", [NGRP, 128, KDH, 2, GC0], FP8,
                                      isOutput=False)
    wq_d = nc.declare_dram_parameter("wq", [L, 128, KDH, 2, D], FP8, isOutput=False)
    wk_d = nc.declare_dram_parameter("wk", [L, 128, KDH, 2, D], FP8, isOutput=False)
    wv_d = nc.declare_dram_parameter("wv", [L, 128, KDH, 2, D], FP8, isOutput=False)
    wo_d = nc.declare_dram_parameter("wo", [L, 128, KDH, 2, D], FP8, isOutput=False)
    w1_d = nc.declare_dram_parameter("w1", [L, NFB, 128, FBT * KD * 128], BF16,
                                     isOutput=False)
    w2_d = nc.declare_dram_parameter("w2", [L, NFB, 128, FBT, D], BF16,
                                     isOutput=False)
    if nonzero_bq:
        bq_d = nc.declare_dram_parameter("bq", [L, 128, MD], F32, isOutput=False)
    if nonzero_b1:
        b1_d = nc.declare_dram_parameter("b1", [L, 128, FT], F32, isOutput=False)
    if nonzero_bo:
        bo_d = nc.declare_dram_parameter("bo_bc", [L, 128, D], F32, isOutput=False)
    if nonzero_b2:
        b2_d = nc.declare_dram_parameter("b2_bc", [L, 128, D], F32, isOutput=False)

    with tile.TileContext(nc) as tc, ExitStack() as top:
        const = top.enter_context(tc.tile_pool(name="const", bufs=1))
        ident = const.tile([128, 128], BF16, name="ident_bf")
        make_identity(nc, ident)

        xpool = top.enter_context(tc.tile_pool(name="xres", bufs=BL))
        stats = top.enter_context(tc.tile_pool(name="stats", bufs=2))
        h1tm_p = top.enter_context(tc.tile_pool(name="h1tm", bufs=4))
        h2tm_p = top.enter_context(tc.tile_pool(name="h2tm", bufs=4))
        h1fm_p = top.enter_context(tc.tile_pool(name="h1fm", bufs=2))
        h2fm_p = top.enter_context(tc.tile_pool(name="h2fm", bufs=2))
        wat_p = top.enter_context(tc.tile_pool(name="wat", bufs=1))
        wffn_p = top.enter_context(tc.tile_pool(name="wffn", bufs=2))
        kv0_p = top.enter_context(tc.tile_pool(name="kv0", bufs=1))
        qg_p = top.enter_context(tc.tile_pool(name="qg", bufs=2))
        kg_p = top.enter_context(tc.tile_pool(name="kg", bufs=1))
        vt_p = top.enter_context(tc.tile_pool(name="vt", bufs=6))
        ag_p = top.enter_context(tc.tile_pool(name="ag", bufs=2))
        praw_p = top.enter_context(tc.tile_pool(name="praw", bufs=2))
        pts_p = top.enter_context(tc.tile_pool(name="pts", bufs=2))
        u_p = top.enter_context(tc.tile_pool(name="u", bufs=9))
        otmp_p = top.enter_context(tc.tile_pool(name="otmp", bufs=2))
        bias_p = top.enter_context(tc.tile_pool(name="bias", bufs=2))
        big = top.enter_context(tc.tile_pool(name="big", bufs=2, space="PSUM"))
        yps = top.enter_context(tc.tile_pool(name="yps", bufs=2, space="PSUM"))
        sa = top.enter_context(tc.tile_pool(name="sa", bufs=2, space="PSUM"))
        tp = top.enter_context(tc.tile_pool(name="tp", bufs=2, space="PSUM"))

        xt = []
        for i in range(BL):
            t_ = xpool.tile([128, D], F32, name=f"x{i}", tag="x")
            nc.sync.dma_start(t_, x_in[i * 128 : (i + 1) * 128, :])
            xt.append(t_)

        def load_attn_weights(l):
            w = {}
            for nm, d_ in (("wq", wq_d), ("wk", wk_d), ("wv", wv_d), ("wo", wo_d)):
                t_ = wat_p.tile([128, KDH, 2, D], FP8, name=f"l{l}_{nm}", tag=nm)
                nc.sync.dma_start(t_, d_[l])
                w[nm] = t_
            return w

        def load_kv0(g):
            t_ = kv0_p.tile([128, KDH, 2, GC0], FP8, name=f"kv0g{g}", tag="kv0")
            nc.sync.dma_start(t_, kv0_d[g])
            return t_

        wts = load_attn_weights(0)
        kv0g0 = load_kv0(0)

        def fm_transpose(src4, dst, evict, tagname):
            for k in range(KD):
                ps = tp.tile([128, 512], BF16, name=f"{tagname}{k}", tag="tp")
                for i in range(4):
                    nc.tensor.transpose(ps[:, i * 128 : (i + 1) * 128],
                                        src4[i][:, k * 128 : (k + 1) * 128], ident)
                evict(k, ps)

        for l in range(n_layers):
            lw = wts
            if nonzero_bq:
                bqt = bias_p.tile([128, MD], F32, name=f"l{l}_bq", tag="bq")
                nc.sync.dma_start(bqt, bq_d[l])

            # ---- LN1 + feature-major fp8 transposes, per group/half ----
            h1fm = []
            for g in range(NGRP):
                h1tm = _layer_norm_half(nc, stats, xt[g * GE : (g + 1) * GE],
                                        h1tm_p, f"l{l}a{g}", "h1")
                fm = h1fm_p.tile([128, KD, GT], FP8, name=f"l{l}h1fm{g}",
                                 tag="h1fm")
                fm_transpose(
                    h1tm, fm,
                    lambda k, ps, fm=fm: nc.vector.tensor_scalar(
                        fm[:, k, :], ps, HSC, None, OP.mult),
                    f"l{l}a{g}tp")
                h1fm.append(fm)

            TCB = C if l == 0 else S
            TCG = GE * TCB

            for g in range(NGRP):
                kvg = (kv0g0 if g == 0 else load_kv0(1)) if l == 0 else None

                # ---- Q projection (DoubleRow fp8) ----
                qg = qg_p.tile([128, MD, GT], BF16, name=f"l{l}g{g}_q", tag="qg")
                for m in range(MD):
                    ps = big.tile([128, 512], F32, name="qps", tag="big")
                    for p in range(KDH):
                        nc.tensor.matmul(
                            ps, lw["wq"][:, p, :, m * 128 : (m + 1) * 128],
                            h1fm[g][:, 2 * p : 2 * p + 2, :],
                            start=(p == 0), stop=(p == KDH - 1), perf_mode=DR)
                    if nonzero_bq:
                        nc.scalar.activation(qg[:, m, :], ps, ACT.Identity,
                                             bias=bqt[:, m : m + 1], scale=SQ[l])
                    else:
                        nc.scalar.activation(qg[:, m, :], ps, ACT.Copy,
                                             scale=SQ[l])
                # ---- K projection ----
                kg = kg_p.tile([128, MD, GC0], BF16, name=f"l{l}g{g}_k", tag="kg")
                for m in range(MD):
                    for n0 in range(0, TCG, 512):
                        n1 = min(n0 + 512, TCG)
                        ps = big.tile([128, 512], F32, name="kps", tag="big")
                        for p in range(KDH):
                            rhs = (kvg[:, p, :, n0:n1] if l == 0
                                   else h1fm[g][:, 2 * p : 2 * p + 2, n0:n1])
                            nc.tensor.matmul(
                                ps[:, : n1 - n0],
                                lw["wk"][:, p, :, m * 128 : (m + 1) * 128],
                                rhs, start=(p == 0), stop=(p == KDH - 1),
                                perf_mode=DR)
                        nc.scalar.activation(kg[:, m, n0:n1], ps[:, : n1 - n0],
                                             ACT.Copy, scale=SK[l])
                # ---- V projection (token-major, per example) ----
                vts = []      # per example: list of (tile, nrows)
                for e in range(GE):
                    segs = []
                    for s0 in range(0, TCB, 128):
                        nr = min(128, TCB - s0)
                        vt = vt_p.tile([128, D], BF16, name=f"l{l}g{g}e{e}v{s0}",
                                       tag="v")
                        for n in range(2):
                            ps = big.tile([128, 512], F32, name="vps", tag="big")
                            for p in range(KDH):
                                if l == 0:
                                    lh = kvg[:, p, :,
                                             e * TCB + s0 : e * TCB + s0 + nr]
                                else:
                                    lh = h1fm[g][:, 2 * p : 2 * p + 2,
                                                 e * 128 : e * 128 + 128]
                                nc.tensor.matmul(
                                    ps[:nr], lh,
                                    lw["wv"][:, p, :, n * 512 : (n + 1) * 512],
                                    start=(p == 0), stop=(p == KDH - 1),
                                    perf_mode=DR)
                            nc.scalar.activation(
                                vt[:nr, n * 512 : (n + 1) * 512], ps[:nr],
                                ACT.Copy, scale=SV[l])
                        segs.append((vt, nr))
                    vts.append(segs)

                # ---- attention smalls ----
                ag = ag_p.tile([128, MD, GT], FP8, name=f"l{l}g{g}_a", tag="ag")
                bh = 4 if l > 0 else 2
                nseg = (TCB + 127) // 128
                for e in range(GE):
                    for hb in range(0, H, bh):
                        praw = praw_p.tile([128, bh, TCB], BF16, name="praw",
                                           tag="praw")
                        for hi in range(bh):
                            h_ = hb + hi
                            po, ch = 64 * (h_ % 2), h_ // 2
                            sp = sa.tile([128, TCB], F32, name="sp", tag="sa")
                            nc.tensor.matmul(
                                sp,
                                qg[po : po + 64, ch, e * 128 : (e + 1) * 128],
                                kg[po : po + 64, ch, e * TCB : (e + 1) * TCB],
                                start=True, stop=True)
                            nc.scalar.activation(praw[:, hi, :], sp, ACT.Exp)
                        ssum = stats.tile([128, bh], F32, name="ssum", tag="ssum")
                        nc.vector.tensor_reduce(ssum, praw, AX, OP.add)
                        rinv = stats.tile([128, bh], F32, name="rinv", tag="rinv")
                        nc.vector.reciprocal(rinv, ssum)
                        pbf = praw
                        nc.vector.tensor_tensor(
                            pbf, praw,
                            rinv[:, :, None].broadcast_to((128, bh, TCB)),
                            OP.mult)
                        if l > 0:
                            # one [128, 4, 128] psum; col order (po,ch) packed
                            col = lambda i: (i % 2) * 2 + i // 2
                            tpp = tp.tile([128, bh, 128], BF16, name="ptp",
                                          tag="tp")
                            for hi in range(bh):
                                nc.tensor.transpose(tpp[:, hi, :],
                                                    pbf[:, hi, :], ident)
                            pts = pts_p.tile([128, bh, 128], BF16, name="pts",
                                             tag="pts")
                            nc.scalar.activation(pts, tpp, ACT.Copy)
                            aps = sa.tile([64, bh, 128], F32, name="aps",
                                          tag="sa")
                            for hi in range(bh):
                                h_ = hb + hi
                                vt, _ = vts[e][0]
                                nc.tensor.matmul(
                                    aps[:, col(hi), :],
                                    vt[:, h_ * 64 : (h_ + 1) * 64],
                                    pts[:, hi, :], start=True, stop=True)
                            ch0 = hb // 2
                            for half in range(2):
                                nc.scalar.activation(
                                    ag[half * 64 : half * 64 + 64,
                                       ch0 : ch0 + 2,
                                       e * 128 : (e + 1) * 128],
                                    aps[:, half * 2 : half * 2 + 2, :],
                                    ACT.Copy, scale=HSC)
                        else:
                            # l0: TCB=196, 2 heads, 2 segs, per-head offset-0 aps
                            tpp = tp.tile([128, nseg, bh, 128], BF16,
                                          name="ptp0", tag="tp")
                            for hi in range(bh):
                                for si in range(nseg):
                                    nr = min(128, TCB - si * 128)
                                    nc.tensor.transpose(
                                        tpp[:nr, si, hi, :],
                                        pbf[:, hi, si * 128 : si * 128 + nr],
                                        ident)
                            pts = pts_p.tile([128, nseg, bh, 128], BF16,
                                             name="pts0", tag="pts")
                            for si in range(nseg):
                                nr = min(128, TCB - si * 128)
                                nc.scalar.activation(pts[:nr, si], tpp[:nr, si],
                                                     ACT.Copy)
                            for hi in range(bh):
                                h_ = hb + hi
                                po, ch = 64 * (h_ % 2), h_ // 2
                                aps = sa.tile([64, 128], F32, name="aps0",
                                              tag="sa")
                                for si in range(nseg):
                                    nr = min(128, TCB - si * 128)
                                    vt, _ = vts[e][si]
                                    nc.tensor.matmul(
                                        aps, vt[:nr, h_ * 64 : (h_ + 1) * 64],
                                        pts[:nr, si, hi, :],
                                        start=(si == 0), stop=(si == nseg - 1))
                                nc.scalar.activation(
                                    ag[po : po + 64, ch,
                                       e * 128 : (e + 1) * 128],
                                    aps, ACT.Copy, scale=HSC)

                # ---- output projection, residual add ----
                for e in range(GE):
                    xi = xt[g * GE + e]
                    for n in range(2):
                        ps = big.tile([128, 512], F32, name="ops", tag="big")
                        for p in range(KDH):
                            nc.tensor.matmul(
                                ps, ag[:, 2 * p : 2 * p + 2,
                                       e * 128 : (e + 1) * 128],
                                lw["wo"][:, p, :, n * 512 : (n + 1) * 512],
                                start=(p == 0), stop=(p == KDH - 1),
                                perf_mode=DR)
                        ot = otmp_p.tile([128, 512], BF16, name="ot", tag="ot")
                        nc.scalar.activation(ot, ps, ACT.Copy, scale=SO[l])
                        nc.vector.tensor_tensor(
                            xi[:, n * 512 : (n + 1) * 512],
                            xi[:, n * 512 : (n + 1) * 512], ot, OP.add)

            # prefetch next layer's attention weights (slots free now)
            if l + 1 < n_layers:
                wts = load_attn_weights(l + 1)

            if nonzero_bo:
                bo_t = bias_p.tile([128, D], F32, name=f"l{l}_bo", tag="bo")
                nc.sync.dma_start(bo_t, bo_d[l])
                for i in range(BL):
                    nc.vector.tensor_tensor(xt[i], xt[i], bo_t, OP.add)

            # ---------------- FFN sublayer ----------------
            if nonzero_b1:
                b1t = bias_p.tile([128, FT], F32, name=f"l{l}_b1", tag="b1")
                nc.sync.dma_start(b1t, b1_d[l])
            for th in range(2):
                h2tm = _layer_norm_half(nc, stats, xt[th * GE : (th + 1) * GE],
                                        h2tm_p, f"l{l}f{th}", "h2")
                h2fm = h2fm_p.tile([128, KD, GT], BF16, name=f"l{l}h2fm{th}",
                                   tag="h2fm")
                fm_transpose(
                    h2tm, h2fm,
                    lambda k, ps, fm=h2fm: nc.vector.tensor_copy(fm[:, k, :], ps),
                    f"l{l}f{th}tp")

                for fbp in range(NFB // 2):
                    fbs = (2 * fbp, 2 * fbp + 1)
                    w1t, w2t = {}, {}
                    for fb in fbs:
                        w1t[fb] = wffn_p.tile([128, FBT * KD * 128], BF16,
                                              name=f"l{l}t{th}w1_{fb}", tag="w1")
                        nc.sync.dma_start(w1t[fb], w1_d[l, fb])
                        w2t[fb] = wffn_p.tile([128, FBT, D], BF16,
                                              name=f"l{l}t{th}w2_{fb}", tag="w2")
                        nc.sync.dma_start(w2t[fb], w2_d[l, fb])
                    uts = []
                    for fb in fbs:
                        for ft_ in range(FBT):
                            ps = big.tile([128, 512], F32, name="ups", tag="big")
                            for k in range(KD):
                                o0 = ft_ * KD * 128 + k * 128
                                nc.tensor.matmul(
                                    ps, w1t[fb][:, o0 : o0 + 128],
                                    h2fm[:, k, :],
                                    start=(k == 0), stop=(k == KD - 1))
                            ut = u_p.tile([128, 512], BF16,
                                          name=f"u{fb}_{ft_}", tag="u")
                            if nonzero_b1:
                                fcol = fb * FBT + ft_
                                nc.scalar.activation(
                                    ut, ps, ACT.Gelu_apprx_tanh,
                                    bias=b1t[:, fcol : fcol + 1])
                            else:
                                nc.scalar.activation(ut, ps, ACT.Gelu_apprx_tanh)
                            uts.append(ut)
                    for m in range(GE):
                        xi = xt[th * GE + m]
                        for n in range(2):
                            yp = yps.tile([128, 512], F32, name="yp", tag="yps")
                            idx = 0
                            for fi, fb in enumerate(fbs):
                                for kf in range(FBT):
                                    nc.tensor.matmul(
                                        yp,
                                        uts[fi * FBT + kf][:, m * 128 :
                                                           (m + 1) * 128],
                                        w2t[fb][:, kf, n * 512 : (n + 1) * 512],
                                        start=(idx == 0),
                                        stop=(idx == 2 * FBT - 1))
                                    idx += 1
                            nc.vector.tensor_tensor(
                                xi[:, n * 512 : (n + 1) * 512],
                                xi[:, n * 512 : (n + 1) * 512], yp, OP.add)
            if nonzero_b2:
                b2t = bias_p.tile([128, D], F32, name=f"l{l}_b2", tag="b2")
                nc.sync.dma_start(b2t, b2_d[l])
                for i in range(BL):
                    nc.vector.tensor_tensor(xt[i], xt[i], b2t, OP.add)

        for i in range(BL):
            nc.sync.dma_start(y_out[i * 128 : (i + 1) * 128, :], xt[i])

    _split_multi_waits(nc)
    return nc


def _pair_layout(w):
    """[D_in, N] -> [128, KDH, 2, N] pair layout for DoubleRow."""
    n = w.shape[1]
    return np.ascontiguousarray(
        w.reshape(KDH, 2, 128, n).transpose(2, 0, 1, 3))


def _fp8_scale(w):
    m = float(np.abs(w).max())
    if m == 0.0:
        return 1.0
    return float(2.0 ** np.floor(np.log2(192.0 / m)))


def prepare_host(inputs, n_layers=L):
    """Fold LN affines + biases into weights; fp8-quantize QKVO; arrange
    DMA-friendly layouts."""
    f32 = np.float32
    Wq = np.asarray(inputs["Wq"], f32)
    Wk = np.asarray(inputs["Wk"], f32)
    Wv = np.asarray(inputs["Wv"], f32)
    Wo = np.asarray(inputs["Wo"], f32)
    W1 = np.asarray(inputs["W1"], f32)
    W2 = np.asarray(inputs["W2"], f32)
    bq = np.asarray(inputs["bq"], f32)
    bv = np.asarray(inputs["bv"], f32)
    bo = np.asarray(inputs["bo"], f32)
    b1 = np.asarray(inputs["b1"], f32)
    b2 = np.asarray(inputs["b2"], f32)
    g1 = np.asarray(inputs["ln1_g"], f32)
    be1 = np.asarray(inputs["ln1_b"], f32)
    g2 = np.asarray(inputs["ln2_g"], f32)
    be2 = np.asarray(inputs["ln2_b"], f32)

    scale = np.float32(1.0 / np.sqrt(DK))
    Wq_e = (g1[:, :, None] * Wq) * scale
    bq_e = (bq + np.einsum("ld,ldo->lo", be1, Wq)) * scale
    Wk_e = Wk.copy()
    Wv_e = Wv.copy()
    bv_e = bv.copy()
    for l in range(1, L):
        Wk_e[l] = g1[l][:, None] * Wk[l]
        Wv_e[l] = g1[l][:, None] * Wv[l]
        bv_e[l] = bv[l] + be1[l] @ Wv[l]
    bo_e = bo + np.einsum("ld,ldo->lo", bv_e, Wo)
    W1_e = g2[:, :, None] * W1
    b1_e = b1 + np.einsum("ld,ldo->lo", be2, W1)

    # fp8 quantization with per-(layer,tensor) power-of-2 scales
    wq8 = np.empty((L, 128, KDH, 2, D), NP_FP8)
    wk8 = np.empty((L, 128, KDH, 2, D), NP_FP8)
    wv8 = np.empty((L, 128, KDH, 2, D), NP_FP8)
    wo8 = np.empty((L, 128, KDH, 2, D), NP_FP8)
    SQ, SK, SV, SO = [], [], [], []
    for l in range(L):
        for w_eff, dst, slist in ((Wq_e[l], wq8, SQ), (Wk_e[l], wk8, SK),
                                  (Wv_e[l], wv8, SV), (Wo[l], wo8, SO)):
            s = _fp8_scale(w_eff)
            dst[l] = _pair_layout(w_eff * s).astype(NP_FP8)
            slist.append(float(1.0 / (HSC * s)))

    # FFN bf16 layouts
    # w1: [Din, F] -> [NFB, 128, FBT*KD*128] with order [r, ft, k, c]
    w1h = np.empty((L, NFB, 128, FBT * KD * 128), NP_BF16)
    w2h = np.empty((L, NFB, 128, FBT, D), NP_BF16)
    for l in range(L):
        a = W1_e[l].reshape(KD, 128, FT, 128).transpose(1, 2, 0, 3)  # r,ft,k,c
        w1h[l] = (a.reshape(128, NFB, FBT, KD * 128).transpose(1, 0, 2, 3)
                  .reshape(NFB, 128, FBT * KD * 128).astype(NP_BF16))
        b_ = W2[l].reshape(NFB, FBT, 128, D).transpose(0, 2, 1, 3)  # fb,r,ft,o
        w2h[l] = b_.astype(NP_BF16)

    meta = {
        "SQ": SQ, "SK": SK, "SV": SV, "SO": SO,
        "nonzero_bq": bool(np.any(bq_e)),
        "nonzero_b1": bool(np.any(b1_e)),
        "nonzero_bo": bool(np.any(bo_e)),
        "nonzero_b2": bool(np.any(b2)),
    }

    host = {"wq": wq8, "wk": wk8, "wv": wv8, "wo": wo8, "w1": w1h, "w2": w2h}
    if meta["nonzero_bq"]:
        host["bq"] = np.ascontiguousarray(
            bq_e.reshape(L, MD, 128).transpose(0, 2, 1))
    if meta["nonzero_b1"]:
        host["b1"] = np.ascontiguousarray(
            b1_e.reshape(L, FT, 128).transpose(0, 2, 1))
    if meta["nonzero_bo"]:
        host["bo_bc"] = np.ascontiguousarray(
            np.broadcast_to(bo_e[:, None, :], (L, 128, D)).astype(f32))
    if meta["nonzero_b2"]:
        host["b2_bc"] = np.ascontiguousarray(
            np.broadcast_to(b2[:, None, :], (L, 128, D)).astype(f32))

    xt = np.asarray(inputs["xt"], f32)
    p_att = np.asarray(inputs["p_att_feats"], f32)
    per_core = []
    for c in range(NCORES):
        xs = np.ascontiguousarray(xt[c * BL : (c + 1) * BL].reshape(T, D))
        kv = np.empty((NGRP, 128, KDH, 2, GC0), NP_FP8)
        for g in range(NGRP):
            blk = p_att[c * BL + g * GE : c * BL + (g + 1) * GE]  # [GE, C, D]
            ft = blk.reshape(GC0, D).T  # [D, GC0]
            kv[g] = _pair_layout(ft * HSC).astype(NP_FP8)
        m = dict(host)
        m["x"] = xs
        m["kv0"] = kv
        per_core.append(m)
    return per_core, meta


def run(inputs, n_layers=L):
    per_core, meta = prepare_host(inputs, n_layers)
    nc = build_program(meta, n_layers)
    res = run_bass_kernel_spmd(nc, per_core, list(range(NCORES)))
    out = np.empty((B, S, D), np.float32)
    for c in range(NCORES):
        out[c * BL : (c + 1) * BL] = res.results[c]["y"].reshape(BL, S, D)
    return out


def kernel(**inputs) -> np.ndarray:
    return run(inputs)


# revision 15
# speedup vs baseline: 1.0373x; 1.0373x over previous
"""Trainium2 Bass kernel for nn_BertAoA_Decoder_Core (6-layer BERT-style encoder,
layer-0 cross-attention to p_att_feats).

Strategy: pure data-parallel over batch across 8 NeuronCores (8 examples/core).
v4: fp8e4m3 DoubleRow matmuls (2x PE rate) for the Q/K/V/O projections with
power-of-2 weight/activation scaling; all projection descales folded into the
softmax Exp scale / ag eviction, so Q/K/V PSUM evictions are plain copies;
softmax denominator via the Exp activation's accum_out (no tensor_reduce);
bf16 FFN weights/activations, fp32 residual stream; FFN weights loaded once
per layer (token-half inner loop); a dedicated PSUM tag for the O-projection
so it never blocks FFN1 slots; weight DMAs on the gpsimd queue.
"""

import sys

sys.path.insert(0, "/opt/trn_rl_repo")

import numpy as np
import ml_dtypes
from contextlib import ExitStack

import concourse.bass as bass
import concourse.mybir as mybir
import concourse.tile as tile
from concourse.masks import make_identity
from concourse.bass_utils import run_bass_kernel_spmd

F32 = mybir.dt.float32
BF16 = mybir.dt.bfloat16
FP8 = mybir.dt.float8e4
AX = mybir.AxisListType.X
OP = mybir.AluOpType
ACT = mybir.ActivationFunctionType
DR = mybir.MatmulPerfMode.DoubleRow

NP_BF16 = ml_dtypes.bfloat16
NP_FP8 = ml_dtypes.float8_e4m3

# Problem constants (hardcoded per contract)
B, S, C, D, H, L, F = 64, 128, 196, 1024, 16, 6, 4096
DK = D // H              # 64
NCORES = 8
BL = B // NCORES         # 8 examples per core
T = BL * S               # 1024 query tokens per core
KD = D // 128            # 8 contraction tiles
KDH = KD // 2            # 4 contraction pair-tiles (DoubleRow)
MD = D // 128            # 8 output tiles
FT = F // 128            # 32 FFN f-tiles
NFB = 8                  # FFN f-blocks
FBT = FT // NFB          # 4 f-tiles per block
NGRP = 2                 # example groups per core (== token halves)
GE = BL // NGRP          # 4 examples per group
GT = GE * S              # 512 tokens per group
GC0 = GE * C             # 784 context tokens per group (layer 0)
LN_EPS = 1e-6
HSC = 16.0               # fp8 activation scale


def _split_multi_waits(nc):
    """This container's walrus accepts only one sync-wait per CTRL instruction;
    hoist extra waits onto preceding NoOps on the same engine."""
    cnt = 0
    for fn in nc.m.functions:
        for bb in fn.blocks:
            new_list = []
            for ins in bb.instructions:
                si = getattr(ins, "sync_info", None)
                ow = getattr(si, "on_wait", None) if si is not None else None
                if ow and len(ow) > 1:
                    for w in ow[:-1]:
                        nop = mybir.InstNoOp(
                            name=f"{ins.name}-wsplit-{cnt}",
                            engine=ins.engine,
                            sync_info=mybir.SyncInfo(on_wait=[w], on_update=[]),
                        )
                        cnt += 1
                        new_list.append(nop)
                    si.on_wait = [ow[-1]]
                new_list.append(ins)
            bb.instructions = new_list
    return cnt


def _newton_rsqrt(nc, pool, v_ap, out_ap, n):
    """out = 1/sqrt(v) elementwise on a small [128, n] fp32 AP, DVE-only."""
    r = pool.tile([128, n], F32, name="rs_r", tag="rs_r")
    t = pool.tile([128, n], F32, name="rs_t", tag="rs_t")
    nc.vector.reciprocal(r, v_ap)
    nc.vector.tensor_scalar(out_ap, r, 0.5, 0.5, OP.mult, OP.add)
    for _ in range(4):
        nc.vector.tensor_tensor(t, out_ap, out_ap, OP.mult)
        nc.vector.tensor_tensor(t, t, v_ap, OP.mult)
        nc.vector.tensor_scalar(t, t, -0.5, 1.5, OP.mult, OP.add)
        nc.vector.tensor_tensor(out_ap, out_ap, t, OP.mult)


def _layer_norm_half(nc, stats_pool, x_tiles, h_pool, tagpfx):
    """Pre-norm (x-mu)*rstd for 4 [128, D] fp32 token-major tiles -> bf16."""
    nb = len(x_tiles)
    stat = stats_pool.tile([128, nb, 12], F32, name=f"{tagpfx}_stat",
                           tag="ln_stat")
    mv = stats_pool.tile([128, nb, 2], F32, name=f"{tagpfx}_mv", tag="ln_mv")
    var = stats_pool.tile([128, nb], F32, name=f"{tagpfx}_var", tag="ln_var")
    rst = stats_pool.tile([128, nb], F32, name=f"{tagpfx}_rst", tag="ln_rst")
    for i in range(nb):
        nc.vector.bn_stats(stat[:, i, 0:6], x_tiles[i][:, 0:512])
        nc.vector.bn_stats(stat[:, i, 6:12], x_tiles[i][:, 512:1024])
        nc.vector.bn_aggr(mv[:, i, :], stat[:, i, :])
    nc.vector.tensor_scalar(var, mv[:, :, 1], LN_EPS, None, OP.add)
    _newton_rsqrt(nc, stats_pool, var, rst, nb)
    h_tiles = []
    for i in range(nb):
        h = h_pool.tile([128, D], BF16, name=f"{tagpfx}_h{i}", tag="htm")
        nc.vector.tensor_scalar(h, x_tiles[i], mv[:, i, 0:1],
                                rst[:, i : i + 1], OP.subtract, OP.mult)
        h_tiles.append(h)
    return h_tiles


def build_program(meta, n_layers=L):
    """meta: dict from prepare_host with descale constants and bias flags."""
    SV, SO, ESK = meta["SV"], meta["SO"], meta["ESK"]
    nonzero_bq, nonzero_b1 = meta["nonzero_bq"], meta["nonzero_b1"]
    nonzero_bo, nonzero_b2 = meta["nonzero_bo"], meta["nonzero_b2"]

    nc = bass.Bass()
    x_in = nc.declare_dram_parameter("x", [T, D], F32, isOutput=False)
    y_out = nc.declare_dram_parameter("y", [T, D], F32, isOutput=True)
    kv0_d = nc.declare_dram_parameter("kv0", [NGRP, 128, KDH, 2, GC0], FP8,
                                      isOutput=False)
    wq_d = nc.declare_dram_parameter("wq", [L, 128, KDH, 2, D], FP8, isOutput=False)
    wk_d = nc.declare_dram_parameter("wk", [L, 128, KDH, 2, D], FP8, isOutput=False)
    wv_d = nc.declare_dram_parameter("wv", [L, 128, KDH, 2, D], FP8, isOutput=False)
    wo_d = nc.declare_dram_parameter("wo", [L, 128, KDH, 2, D], FP8, isOutput=False)
    w1_d = nc.declare_dram_parameter("w1", [L, NFB, 128, FBT * KD * 128], BF16,
                                     isOutput=False)
    w2_d = nc.declare_dram_parameter("w2", [L, NFB, 128, FBT, D], BF16,
                                     isOutput=False)
    if nonzero_bq:
        bq_d = nc.declare_dram_parameter("bq", [L, 128, MD], F32, isOutput=False)
    if nonzero_b1:
        b1_d = nc.declare_dram_parameter("b1", [L, 128, FT], F32, isOutput=False)
    if nonzero_bo:
        bo_d = nc.declare_dram_parameter("bo_bc", [L, 128, D], F32, isOutput=False)
    if nonzero_b2:
        b2_d = nc.declare_dram_parameter("b2_bc", [L, 128, D], F32, isOutput=False)

    with tile.TileContext(nc) as tc, ExitStack() as top:
        const = top.enter_context(tc.tile_pool(name="const", bufs=1))
        ident = const.tile([128, 128], BF16, name="ident_bf")
        make_identity(nc, ident)

        xpool = top.enter_context(tc.tile_pool(name="xres", bufs=BL))
        stats = top.enter_context(tc.tile_pool(name="stats", bufs=2))
        htm_p = top.enter_context(tc.tile_pool(name="htm", bufs=4))
        h1fm_p = top.enter_context(tc.tile_pool(name="h1fm", bufs=2))
        h2fm_p = top.enter_context(tc.tile_pool(name="h2fm", bufs=2))
        wat_p = top.enter_context(tc.tile_pool(name="wat", bufs=1))
        wffn_p = top.enter_context(tc.tile_pool(name="wffn", bufs=2))
        kv0_p = top.enter_context(tc.tile_pool(name="kv0", bufs=1))
        qg_p = top.enter_context(tc.tile_pool(name="qg", bufs=2))
        kg_p = top.enter_context(tc.tile_pool(name="kg", bufs=2))
        vt_p = top.enter_context(tc.tile_pool(name="vt", bufs=6))
        ag_p = top.enter_context(tc.tile_pool(name="ag", bufs=1))
        praw_p = top.enter_context(tc.tile_pool(name="praw", bufs=3))
        pts_p = top.enter_context(tc.tile_pool(name="pts", bufs=2))
        u_p = top.enter_context(tc.tile_pool(name="u", bufs=8))
        otmp_p = top.enter_context(tc.tile_pool(name="otmp", bufs=1))
        bias_p = top.enter_context(tc.tile_pool(name="bias", bufs=2))
        big = top.enter_context(tc.tile_pool(name="big", bufs=2, space="PSUM"))
        yps = top.enter_context(tc.tile_pool(name="yps", bufs=1, space="PSUM"))
        ops_p = top.enter_context(tc.tile_pool(name="ops", bufs=1, space="PSUM"))
        sa = top.enter_context(tc.tile_pool(name="sa", bufs=2, space="PSUM"))
        tp = top.enter_context(tc.tile_pool(name="tp", bufs=2, space="PSUM"))

        xt = []
        for i in range(BL):
            t_ = xpool.tile([128, D], F32, name=f"x{i}", tag="x")
            nc.sync.dma_start(t_, x_in[i * 128 : (i + 1) * 128, :])
            xt.append(t_)

        def load_attn_weights(l):
            w = {}
            for nm, d_ in (("wq", wq_d), ("wk", wk_d), ("wv", wv_d), ("wo", wo_d)):
                t_ = wat_p.tile([128, KDH, 2, D], FP8, name=f"l{l}_{nm}", tag=nm)
                nc.gpsimd.dma_start(t_, d_[l])
                w[nm] = t_
            return w

        def load_kv0(g):
            t_ = kv0_p.tile([128, KDH, 2, GC0], FP8, name=f"kv0g{g}", tag="kv0")
            nc.sync.dma_start(t_, kv0_d[g])
            return t_

        wts = load_attn_weights(0)
        kv0g0 = load_kv0(0)

        def fm_transpose(src4, dst, evict, tagname):
            for k in range(KD):
                ps = tp.tile([128, 512], BF16, name=f"{tagname}{k}", tag="tp")
                for i in range(4):
                    nc.tensor.transpose(ps[:, i * 128 : (i + 1) * 128],
                                        src4[i][:, k * 128 : (k + 1) * 128], ident)
                evict(k, ps)

        for l in range(n_layers):
            lw = wts
            if nonzero_bq:
                bqt = bias_p.tile([128, MD], F32, name=f"l{l}_bq", tag="bq")
                nc.sync.dma_start(bqt, bq_d[l])

            # ---- LN1 + feature-major fp8 transposes, per group/half ----
            h1fm = []
            for g in range(NGRP):
                h1tm = _layer_norm_half(nc, stats, xt[g * GE : (g + 1) * GE],
                                        htm_p, f"l{l}a{g}")
                fm = h1fm_p.tile([128, KD, GT], FP8, name=f"l{l}h1fm{g}",
                                 tag="h1fm")
                fm_transpose(
                    h1tm, fm,
                    lambda k, ps, fm=fm: nc.scalar.activation(
                        fm[:, k, :], ps, ACT.Copy, scale=HSC),
                    f"l{l}a{g}tp")
                h1fm.append(fm)

            TCB = C if l == 0 else S
            TCG = GE * TCB

            h2fms = []
            for g in range(NGRP):
                kvg = (kv0g0 if g == 0 else load_kv0(1)) if l == 0 else None

                # ---- Q projection (DoubleRow fp8, raw eviction) ----
                qg = qg_p.tile([128, MD, GT], BF16, name=f"l{l}g{g}_q", tag="qg")
                for m in range(MD):
                    ps = big.tile([128, 512], F32, name="qps", tag="big")
                    for p in range(KDH):
                        nc.tensor.matmul(
                            ps, lw["wq"][:, p, :, m * 128 : (m + 1) * 128],
                            h1fm[g][:, 2 * p : 2 * p + 2, :],
                            start=(p == 0), stop=(p == KDH - 1), perf_mode=DR)
                    if nonzero_bq:
                        nc.scalar.activation(qg[:, m, :], ps, ACT.Identity,
                                             bias=bqt[:, m : m + 1])
                    else:
                        nc.scalar.activation(qg[:, m, :], ps, ACT.Copy)
                # ---- K projection ----
                kg = kg_p.tile([128, MD, GC0], BF16, name=f"l{l}g{g}_k", tag="kg")
                for m in range(MD):
                    for n0 in range(0, TCG, 512):
                        n1 = min(n0 + 512, TCG)
                        ps = big.tile([128, 512], F32, name="kps", tag="big")
                        for p in range(KDH):
                            rhs = (kvg[:, p, :, n0:n1] if l == 0
                                   else h1fm[g][:, 2 * p : 2 * p + 2, n0:n1])
                            nc.tensor.matmul(
                                ps[:, : n1 - n0],
                                lw["wk"][:, p, :, m * 128 : (m + 1) * 128],
                                rhs, start=(p == 0), stop=(p == KDH - 1),
                                perf_mode=DR)
                        nc.scalar.activation(kg[:, m, n0:n1], ps[:, : n1 - n0],
                                             ACT.Copy)
                # ---- V projection (token-major, per example, raw) ----
                vts = []
                for e in range(GE):
                    segs = []
                    for s0 in range(0, TCB, 128):
                        nr = min(128, TCB - s0)
                        vt = vt_p.tile([128, D], BF16, name=f"l{l}g{g}e{e}v{s0}",
                                       tag="v")
                        for n in range(2):
                            ps = big.tile([128, 512], F32, name="vps", tag="big")
                            for p in range(KDH):
                                if l == 0:
                                    lh = kvg[:, p, :,
                                             e * TCB + s0 : e * TCB + s0 + nr]
                                else:
                                    lh = h1fm[g][:, 2 * p : 2 * p + 2,
                                                 e * 128 : e * 128 + 128]
                                nc.tensor.matmul(
                                    ps[:nr], lh,
                                    lw["wv"][:, p, :, n * 512 : (n + 1) * 512],
                                    start=(p == 0), stop=(p == KDH - 1),
                                    perf_mode=DR)
                            nc.scalar.activation(
                                vt[:nr, n * 512 : (n + 1) * 512], ps[:nr],
                                ACT.Copy)
                        segs.append((vt, nr))
                    vts.append(segs)

                # ---- attention smalls ----
                ag = ag_p.tile([128, MD, GT], FP8, name=f"l{l}g{g}_a", tag="ag")
                bh = 4 if l > 0 else 2
                nseg = (TCB + 127) // 128
                for e in range(GE):
                    for hb in range(0, H, bh):
                        praw = praw_p.tile([128, bh, TCB], BF16, name="praw",
                                           tag="praw")
                        zrow = stats.tile([128, bh], F32, name="zrow",
                                          tag="zrow")
                        for hi in range(bh):
                            h_ = hb + hi
                            po, ch = 64 * (h_ % 2), h_ // 2
                            sp = sa.tile([128, TCB], F32, name="sp", tag="sa")
                            nc.tensor.matmul(
                                sp,
                                qg[po : po + 64, ch, e * 128 : (e + 1) * 128],
                                kg[po : po + 64, ch, e * TCB : (e + 1) * TCB],
                                start=True, stop=True)
                            nc.scalar.activation(praw[:, hi, :], sp, ACT.Exp,
                                                 scale=ESK[l],
                                                 accum_out=zrow[:, hi : hi + 1])
                        rinv = stats.tile([128, bh], F32, name="rinv", tag="rinv")
                        nc.vector.reciprocal(rinv, zrow)
                        pbf = praw
                        nc.vector.tensor_tensor(
                            pbf, praw,
                            rinv[:, :, None].broadcast_to((128, bh, TCB)),
                            OP.mult)
                        if l > 0:
                            col = lambda i: (i % 2) * 2 + i // 2
                            tpp = tp.tile([128, bh, 128], BF16, name="ptp",
                                          tag="tp")
                            for hi in range(bh):
                                nc.tensor.transpose(tpp[:, hi, :],
                                                    pbf[:, hi, :], ident)
                            pts = pts_p.tile([128, bh, 128], BF16, name="pts",
                                             tag="pts")
                            nc.vector.tensor_copy(pts, tpp)
                            aps = sa.tile([64, bh, 128], F32, name="aps",
                                          tag="sa")
                            for hi in range(bh):
                                h_ = hb + hi
                                vt, _ = vts[e][0]
                                nc.tensor.matmul(
                                    aps[:, col(hi), :],
                                    vt[:, h_ * 64 : (h_ + 1) * 64],
                                    pts[:, hi, :], start=True, stop=True)
                            ch0 = hb // 2
                            for half in range(2):
                                nc.vector.tensor_scalar(
                                    ag[half * 64 : half * 64 + 64,
                                       ch0 : ch0 + 2,
                                       e * 128 : (e + 1) * 128],
                                    aps[:, half * 2 : half * 2 + 2, :],
                                    HSC * SV[l], None, OP.mult)
                        else:
                            tpp = tp.tile([128, nseg, bh, 128], BF16,
                                          name="ptp0", tag="tp")
                            for hi in range(bh):
                                for si in range(nseg):
                                    nr = min(128, TCB - si * 128)
                                    nc.tensor.transpose(
                                        tpp[:nr, si, hi, :],
                                        pbf[:, hi, si * 128 : si * 128 + nr],
                                        ident)
                            pts = pts_p.tile([128, nseg, bh, 128], BF16,
                                             name="pts0", tag="pts")
                            for si in range(nseg):
                                nr = min(128, TCB - si * 128)
                                nc.vector.tensor_copy(pts[:nr, si], tpp[:nr, si])
                            for hi in range(bh):
                                h_ = hb + hi
                                po, ch = 64 * (h_ % 2), h_ // 2
                                aps = sa.tile([64, 128], F32, name="aps0",
                                              tag="sa")
                                for si in range(nseg):
                                    nr = min(128, TCB - si * 128)
                                    vt, _ = vts[e][si]
                                    nc.tensor.matmul(
                                        aps, vt[:nr, h_ * 64 : (h_ + 1) * 64],
                                        pts[:nr, si, hi, :],
                                        start=(si == 0), stop=(si == nseg - 1))
                                nc.vector.tensor_scalar(
                                    ag[po : po + 64, ch,
                                       e * 128 : (e + 1) * 128],
                                    aps, HSC * SV[l], None, OP.mult)

                # ---- output projection (own PSUM tag), residual add ----
                for e in range(GE):
                    xi = xt[g * GE + e]
                    for n in range(2):
                        ps = ops_p.tile([128, 512], F32, name="ops", tag="ops")
                        for p in range(KDH):
                            nc.tensor.matmul(
                                ps, ag[:, 2 * p : 2 * p + 2,
                                       e * 128 : (e + 1) * 128],
                                lw["wo"][:, p, :, n * 512 : (n + 1) * 512],
                                start=(p == 0), stop=(p == KDH - 1),
                                perf_mode=DR)
                        ot = otmp_p.tile([128, 512], BF16, name="ot", tag="ot")
                        nc.scalar.activation(ot, ps, ACT.Copy, scale=SO[l])
                        nc.vector.tensor_tensor(
                            xi[:, n * 512 : (n + 1) * 512],
                            xi[:, n * 512 : (n + 1) * 512], ot, OP.add)

                # LN2 + transposes for this token half (overlaps group g+1)
                if not nonzero_bo:
                    h2tm = _layer_norm_half(nc, stats,
                                            xt[g * GE : (g + 1) * GE],
                                            htm_p, f"l{l}f{g}")
                    fm = h2fm_p.tile([128, KD, GT], BF16, name=f"l{l}h2fm{g}",
                                     tag="h2fm")
                    fm_transpose(
                        h2tm, fm,
                        lambda k, ps, fm=fm: nc.scalar.activation(
                            fm[:, k, :], ps, ACT.Copy),
                        f"l{l}f{g}tp")
                    h2fms.append(fm)

            # prefetch next layer's attention weights (slots free now)
            if l + 1 < n_layers:
                wts = load_attn_weights(l + 1)

            if nonzero_bo:
                bo_t = bias_p.tile([128, D], F32, name=f"l{l}_bo", tag="bo")
                nc.sync.dma_start(bo_t, bo_d[l])
                for i in range(BL):
                    nc.vector.tensor_tensor(xt[i], xt[i], bo_t, OP.add)
                for th in range(2):
                    h2tm = _layer_norm_half(nc, stats,
                                            xt[th * GE : (th + 1) * GE],
                                            htm_p, f"l{l}f{th}")
                    fm = h2fm_p.tile([128, KD, GT], BF16,
                                     name=f"l{l}h2fm{th}", tag="h2fm")
                    fm_transpose(
                        h2tm, fm,
                        lambda k, ps, fm=fm: nc.scalar.activation(
                            fm[:, k, :], ps, ACT.Copy),
                        f"l{l}f{th}tp")
                    h2fms.append(fm)

            # ---------------- FFN sublayer ----------------
            if nonzero_b1:
                b1t = bias_p.tile([128, FT], F32, name=f"l{l}_b1", tag="b1")
                nc.sync.dma_start(b1t, b1_d[l])
            for fbp in range(NFB // 2):
                fbs = (2 * fbp, 2 * fbp + 1)
                w1t, w2t = {}, {}
                for fb in fbs:
                    w1t[fb] = wffn_p.tile([128, FBT * KD * 128], BF16,
                                          name=f"l{l}w1_{fb}", tag="w1")
                    nc.gpsimd.dma_start(w1t[fb], w1_d[l, fb])
                    w2t[fb] = wffn_p.tile([128, FBT, D], BF16,
                                          name=f"l{l}w2_{fb}", tag="w2")
                    nc.gpsimd.dma_start(w2t[fb], w2_d[l, fb])
                for th in range(2):
                    h2fm = h2fms[th]
                    uts = []
                    for fb in fbs:
                        for ft_ in range(FBT):
                            ps = big.tile([128, 512], F32, name="ups", tag="big")
                            for k in range(KD):
                                o0 = ft_ * KD * 128 + k * 128
                                nc.tensor.matmul(
                                    ps, w1t[fb][:, o0 : o0 + 128],
                                    h2fm[:, k, :],
                                    start=(k == 0), stop=(k == KD - 1))
                            ut = u_p.tile([128, 512], BF16,
                                          name=f"u{fb}_{ft_}", tag="u")
                            if nonzero_b1:
                                fcol = fb * FBT + ft_
                                nc.scalar.activation(
                                    ut, ps, ACT.Gelu_apprx_tanh,
                                    bias=b1t[:, fcol : fcol + 1])
                            else:
                                nc.scalar.activation(ut, ps,
                                                     ACT.Gelu_apprx_tanh)
                            uts.append(ut)
                    for m in range(GE):
                        xi = xt[th * GE + m]
                        for n in range(2):
                            yp = yps.tile([128, 512], F32, name="yp", tag="yps")
                            idx = 0
                            for fi, fb in enumerate(fbs):
                                for kf in range(FBT):
                                    nc.tensor.matmul(
                                        yp,
                                        uts[fi * FBT + kf][:, m * 128 :
                                                           (m + 1) * 128],
                                        w2t[fb][:, kf, n * 512 : (n + 1) * 512],
                                        start=(idx == 0),
                                        stop=(idx == 2 * FBT - 1))
                                    idx += 1
                            nc.vector.tensor_tensor(
                                xi[:, n * 512 : (n + 1) * 512],
                                xi[:, n * 512 : (n + 1) * 512], yp, OP.add)
            if nonzero_b2:
                b2t = bias_p.tile([128, D], F32, name=f"l{l}_b2", tag="b2")
                nc.sync.dma_start(b2t, b2_d[l])
                for i in range(BL):
                    nc.vector.tensor_tensor(xt[i], xt[i], b2t, OP.add)

        for i in range(BL):
            nc.sync.dma_start(y_out[i * 128 : (i + 1) * 128, :], xt[i])

    _split_multi_waits(nc)
    return nc


def _pair_layout(w):
    """[D_in, N] -> [128, KDH, 2, N] pair layout for DoubleRow."""
    n = w.shape[1]
    return np.ascontiguousarray(
        w.reshape(KDH, 2, 128, n).transpose(2, 0, 1, 3))


def _fp8_scale(w):
    m = float(np.abs(w).max())
    if m == 0.0:
        return 1.0
    return float(2.0 ** np.floor(np.log2(192.0 / m)))


def prepare_host(inputs, n_layers=L):
    """Fold LN affines + biases into weights; fp8-quantize QKVO; arrange
    DMA-friendly layouts."""
    f32 = np.float32
    Wq = np.asarray(inputs["Wq"], f32)
    Wk = np.asarray(inputs["Wk"], f32)
    Wv = np.asarray(inputs["Wv"], f32)
    Wo = np.asarray(inputs["Wo"], f32)
    W1 = np.asarray(inputs["W1"], f32)
    W2 = np.asarray(inputs["W2"], f32)
    bq = np.asarray(inputs["bq"], f32)
    bv = np.asarray(inputs["bv"], f32)
    bo = np.asarray(inputs["bo"], f32)
    b1 = np.asarray(inputs["b1"], f32)
    b2 = np.asarray(inputs["b2"], f32)
    g1 = np.asarray(inputs["ln1_g"], f32)
    be1 = np.asarray(inputs["ln1_b"], f32)
    g2 = np.asarray(inputs["ln2_g"], f32)
    be2 = np.asarray(inputs["ln2_b"], f32)

    scale = np.float32(1.0 / np.sqrt(DK))
    Wq_e = (g1[:, :, None] * Wq) * scale
    bq_e = (bq + np.einsum("ld,ldo->lo", be1, Wq)) * scale
    Wk_e = Wk.copy()
    Wv_e = Wv.copy()
    bv_e = bv.copy()
    for l in range(1, L):
        Wk_e[l] = g1[l][:, None] * Wk[l]
        Wv_e[l] = g1[l][:, None] * Wv[l]
        bv_e[l] = bv[l] + be1[l] @ Wv[l]
    bo_e = bo + np.einsum("ld,ldo->lo", bv_e, Wo)
    W1_e = g2[:, :, None] * W1
    b1_e = b1 + np.einsum("ld,ldo->lo", be2, W1)

    wq8 = np.empty((L, 128, KDH, 2, D), NP_FP8)
    wk8 = np.empty((L, 128, KDH, 2, D), NP_FP8)
    wv8 = np.empty((L, 128, KDH, 2, D), NP_FP8)
    wo8 = np.empty((L, 128, KDH, 2, D), NP_FP8)
    SV, SO, ESK = [], [], []
    for l in range(L):
        sq = _fp8_scale(Wq_e[l])
        wq8[l] = _pair_layout(Wq_e[l] * sq).astype(NP_FP8)
        sk = _fp8_scale(Wk_e[l])
        wk8[l] = _pair_layout(Wk_e[l] * sk).astype(NP_FP8)
        sv = _fp8_scale(Wv_e[l])
        wv8[l] = _pair_layout(Wv_e[l] * sv).astype(NP_FP8)
        so = _fp8_scale(Wo[l])
        wo8[l] = _pair_layout(Wo[l] * so).astype(NP_FP8)
        # scores psum = (16 sq qhat)·(16 sk k) -> exp scale
        ESK.append(float(1.0 / (HSC * HSC * sq * sk)))
        # vt holds 16·sv·v; ag = aps * HSC*SV = HSC·a
        SV.append(float(1.0 / (HSC * sv)))
        # O psum = (HSC a)·(so Wo) -> ot scale
        SO.append(float(1.0 / (HSC * so)))

    w1h = np.empty((L, NFB, 128, FBT * KD * 128), NP_BF16)
    w2h = np.empty((L, NFB, 128, FBT, D), NP_BF16)
    for l in range(L):
        a = W1_e[l].reshape(KD, 128, FT, 128).transpose(1, 2, 0, 3)  # r,ft,k,c
        w1h[l] = (a.reshape(128, NFB, FBT, KD * 128).transpose(1, 0, 2, 3)
                  .reshape(NFB, 128, FBT * KD * 128).astype(NP_BF16))
        b_ = W2[l].reshape(NFB, FBT, 128, D).transpose(0, 2, 1, 3)  # fb,r,ft,o
        w2h[l] = b_.astype(NP_BF16)

    meta = {
        "SV": SV, "SO": SO, "ESK": ESK,
        "nonzero_bq": bool(np.any(bq_e)),
        "nonzero_b1": bool(np.any(b1_e)),
        "nonzero_bo": bool(np.any(bo_e)),
        "nonzero_b2": bool(np.any(b2)),
    }

    host = {"wq": wq8, "wk": wk8, "wv": wv8, "wo": wo8, "w1": w1h, "w2": w2h}
    if meta["nonzero_bq"]:
        host["bq"] = np.ascontiguousarray(
            bq_e.reshape(L, MD, 128).transpose(0, 2, 1))
    if meta["nonzero_b1"]:
        host["b1"] = np.ascontiguousarray(
            b1_e.reshape(L, FT, 128).transpose(0, 2, 1))
    if meta["nonzero_bo"]:
        host["bo_bc"] = np.ascontiguousarray(
            np.broadcast_to(bo_e[:, None, :], (L, 128, D)).astype(f32))
    if meta["nonzero_b2"]:
        host["b2_bc"] = np.ascontiguousarray(
            np.broadcast_to(b2[:, None, :], (L, 128, D)).astype(f32))

    xt = np.asarray(inputs["xt"], f32)
    p_att = np.asarray(inputs["p_att_feats"], f32)
    per_core = []
    for c in range(NCORES):
        xs = np.ascontiguousarray(xt[c * BL : (c + 1) * BL].reshape(T, D))
        kv = np.empty((NGRP, 128, KDH, 2, GC0), NP_FP8)
        for g in range(NGRP):
            blk = p_att[c * BL + g * GE : c * BL + (g + 1) * GE]  # [GE, C, D]
            ft = blk.reshape(GC0, D).T  # [D, GC0]
            kv[g] = _pair_layout(ft * HSC).astype(NP_FP8)
        m = dict(host)
        m["x"] = xs
        m["kv0"] = kv
        per_core.append(m)
    return per_core, meta


def run(inputs, n_layers=L):
    per_core, meta = prepare_host(inputs, n_layers)
    nc = build_program(meta, n_layers)
    res = run_bass_kernel_spmd(nc, per_core, list(range(NCORES)))
    out = np.empty((B, S, D), np.float32)
    for c in range(NCORES):
        out[c * BL : (c + 1) * BL] = res.results[c]["y"].reshape(BL, S, D)
    return out


def kernel(**inputs) -> np.ndarray:
    return run(inputs)


# revision 18
# speedup vs baseline: 1.2194x; 1.1755x over previous
"""Trainium2 Bass kernel for nn_BertAoA_Decoder_Core (6-layer BERT-style encoder,
layer-0 cross-attention to p_att_feats).

Strategy: pure data-parallel over batch across 8 NeuronCores (8 examples/core).
v4: fp8e4m3 DoubleRow matmuls (2x PE rate) for the Q/K/V/O projections with
power-of-2 weight/activation scaling; all projection descales folded into the
softmax Exp scale / ag eviction, so Q/K/V PSUM evictions are plain copies;
softmax denominator via the Exp activation's accum_out (no tensor_reduce);
bf16 FFN weights/activations, fp32 residual stream; FFN weights loaded once
per layer (token-half inner loop); a dedicated PSUM tag for the O-projection
so it never blocks FFN1 slots; weight DMAs on the gpsimd queue.
"""

import sys

sys.path.insert(0, "/opt/trn_rl_repo")

import numpy as np
import ml_dtypes
from contextlib import ExitStack

import concourse.bass as bass
import concourse.mybir as mybir
import concourse.tile as tile
from concourse.masks import make_identity
from concourse.bass_utils import run_bass_kernel_spmd

F32 = mybir.dt.float32
BF16 = mybir.dt.bfloat16
FP8 = mybir.dt.float8e4
AX = mybir.AxisListType.X
OP = mybir.AluOpType
ACT = mybir.ActivationFunctionType
DR = mybir.MatmulPerfMode.DoubleRow

NP_BF16 = ml_dtypes.bfloat16
NP_FP8 = ml_dtypes.float8_e4m3

# Problem constants (hardcoded per contract)
B, S, C, D, H, L, F = 64, 128, 196, 1024, 16, 6, 4096
DK = D // H              # 64
NCORES = 8
BL = B // NCORES         # 8 examples per core
T = BL * S               # 1024 query tokens per core
KD = D // 128            # 8 contraction tiles
KDH = KD // 2            # 4 contraction pair-tiles (DoubleRow)
MD = D // 128            # 8 output tiles
FT = F // 128            # 32 FFN f-tiles
NFB = 8                  # FFN f-blocks
FBT = FT // NFB          # 4 f-tiles per block
NGRP = 2                 # example groups per core (== token halves)
GE = BL // NGRP          # 4 examples per group
GT = GE * S              # 512 tokens per group
GC0 = GE * C             # 784 context tokens per group (layer 0)
LN_EPS = 1e-6
HSC = 16.0               # fp8 activation scale


def _split_multi_waits(nc):
    """This container's walrus accepts only one sync-wait per CTRL instruction;
    hoist extra waits onto preceding NoOps on the same engine."""
    cnt = 0
    for fn in nc.m.functions:
        for bb in fn.blocks:
            new_list = []
            for ins in bb.instructions:
                si = getattr(ins, "sync_info", None)
                ow = getattr(si, "on_wait", None) if si is not None else None
                if ow and len(ow) > 1:
                    for w in ow[:-1]:
                        nop = mybir.InstNoOp(
                            name=f"{ins.name}-wsplit-{cnt}",
                            engine=ins.engine,
                            sync_info=mybir.SyncInfo(on_wait=[w], on_update=[]),
                        )
                        cnt += 1
                        new_list.append(nop)
                    si.on_wait = [ow[-1]]
                new_list.append(ins)
            bb.instructions = new_list
    return cnt


def _newton_rsqrt(nc, pool, v_ap, out_ap, n):
    """out = 1/sqrt(v) elementwise on a small [128, n] fp32 AP, DVE-only."""
    r = pool.tile([128, n], F32, name="rs_r", tag="rs_r")
    t = pool.tile([128, n], F32, name="rs_t", tag="rs_t")
    nc.vector.reciprocal(r, v_ap)
    nc.vector.tensor_scalar(out_ap, r, 0.5, 0.5, OP.mult, OP.add)
    for _ in range(4):
        nc.vector.tensor_tensor(t, out_ap, out_ap, OP.mult)
        nc.vector.tensor_tensor(t, t, v_ap, OP.mult)
        nc.vector.tensor_scalar(t, t, -0.5, 1.5, OP.mult, OP.add)
        nc.vector.tensor_tensor(out_ap, out_ap, t, OP.mult)


def _layer_norm_half(nc, stats_pool, x_tiles, h_pool, tagpfx):
    """Pre-norm (x-mu)*rstd for 4 [128, D] fp32 token-major tiles -> bf16."""
    nb = len(x_tiles)
    stat = stats_pool.tile([128, nb, 12], F32, name=f"{tagpfx}_stat",
                           tag="ln_stat")
    mv = stats_pool.tile([128, nb, 2], F32, name=f"{tagpfx}_mv", tag="ln_mv")
    var = stats_pool.tile([128, nb], F32, name=f"{tagpfx}_var", tag="ln_var")
    rst = stats_pool.tile([128, nb], F32, name=f"{tagpfx}_rst", tag="ln_rst")
    for i in range(nb):
        nc.vector.bn_stats(stat[:, i, 0:6], x_tiles[i][:, 0:512])
        nc.vector.bn_stats(stat[:, i, 6:12], x_tiles[i][:, 512:1024])
        nc.vector.bn_aggr(mv[:, i, :], stat[:, i, :])
    nc.vector.tensor_scalar(var, mv[:, :, 1], LN_EPS, None, OP.add)
    _newton_rsqrt(nc, stats_pool, var, rst, nb)
    h_tiles = []
    for i in range(nb):
        h = h_pool.tile([128, D], BF16, name=f"{tagpfx}_h{i}", tag="htm")
        nc.vector.tensor_scalar(h, x_tiles[i], mv[:, i, 0:1],
                                rst[:, i : i + 1], OP.subtract, OP.mult)
        h_tiles.append(h)
    return h_tiles


def build_program(meta, n_layers=L):
    """meta: dict from prepare_host with descale constants and bias flags."""
    SV, SO, ESK = meta["SV"], meta["SO"], meta["ESK"]
    nonzero_bq, nonzero_b1 = meta["nonzero_bq"], meta["nonzero_b1"]
    nonzero_bo, nonzero_b2 = meta["nonzero_bo"], meta["nonzero_b2"]

    nc = bass.Bass()
    x_in = nc.declare_dram_parameter("x", [T, D], F32, isOutput=False)
    y_out = nc.declare_dram_parameter("y", [T, D], F32, isOutput=True)
    kv0_d = nc.declare_dram_parameter("kv0", [NGRP, 128, KDH, 2, GC0], FP8,
                                      isOutput=False)
    wq_d = nc.declare_dram_parameter("wq", [L, 128, KDH, 2, D], FP8, isOutput=False)
    wk_d = nc.declare_dram_parameter("wk", [L, 128, KDH, 2, D], FP8, isOutput=False)
    wv_d = nc.declare_dram_parameter("wv", [L, 128, KDH, 2, D], FP8, isOutput=False)
    wo_d = nc.declare_dram_parameter("wo", [L, 128, KDH, 2, D], FP8, isOutput=False)
    w1_d = nc.declare_dram_parameter("w1", [L, NFB, 128, FBT * KD * 128], BF16,
                                     isOutput=False)
    w2_d = nc.declare_dram_parameter("w2", [L, NFB, 128, FBT, D], BF16,
                                     isOutput=False)
    if nonzero_bq:
        bq_d = nc.declare_dram_parameter("bq", [L, 128, MD], F32, isOutput=False)
    if nonzero_b1:
        b1_d = nc.declare_dram_parameter("b1", [L, 128, FT], F32, isOutput=False)
    if nonzero_bo:
        bo_d = nc.declare_dram_parameter("bo_bc", [L, 128, D], F32, isOutput=False)
    if nonzero_b2:
        b2_d = nc.declare_dram_parameter("b2_bc", [L, 128, D], F32, isOutput=False)

    with tile.TileContext(nc) as tc, ExitStack() as top:
        const = top.enter_context(tc.tile_pool(name="const", bufs=1))
        ident = const.tile([128, 128], BF16, name="ident_bf")
        make_identity(nc, ident)

        xpool = top.enter_context(tc.tile_pool(name="xres", bufs=BL))
        stats = top.enter_context(tc.tile_pool(name="stats", bufs=2))
        htm_p = top.enter_context(tc.tile_pool(name="htm", bufs=4))
        h1fm_p = top.enter_context(tc.tile_pool(name="h1fm", bufs=2))
        h2fm_p = top.enter_context(tc.tile_pool(name="h2fm", bufs=2))
        wat_p = top.enter_context(tc.tile_pool(name="wat", bufs=1))
        wffn_p = top.enter_context(tc.tile_pool(name="wffn", bufs=2))
        kv0_p = top.enter_context(tc.tile_pool(name="kv0", bufs=1))
        qg_p = top.enter_context(tc.tile_pool(name="qg", bufs=2))
        kg_p = top.enter_context(tc.tile_pool(name="kg", bufs=2))
        vt_p = top.enter_context(tc.tile_pool(name="vt", bufs=6))
        ag_p = top.enter_context(tc.tile_pool(name="ag", bufs=1))
        praw_p = top.enter_context(tc.tile_pool(name="praw", bufs=3))
        pts_p = top.enter_context(tc.tile_pool(name="pts", bufs=2))
        u_p = top.enter_context(tc.tile_pool(name="u", bufs=8))
        otmp_p = top.enter_context(tc.tile_pool(name="otmp", bufs=1))
        bias_p = top.enter_context(tc.tile_pool(name="bias", bufs=2))
        big = top.enter_context(tc.tile_pool(name="big", bufs=2, space="PSUM"))
        fps = top.enter_context(tc.tile_pool(name="fps", bufs=3, space="PSUM"))
        sa = top.enter_context(tc.tile_pool(name="sa", bufs=2, space="PSUM"))
        tp = top.enter_context(tc.tile_pool(name="tp", bufs=1, space="PSUM"))

        xt = []
        for i in range(BL):
            t_ = xpool.tile([128, D], F32, name=f"x{i}", tag="x")
            nc.sync.dma_start(t_, x_in[i * 128 : (i + 1) * 128, :])
            xt.append(t_)

        def load_attn_weights(l):
            w = {}
            for nm, d_ in (("wq", wq_d), ("wk", wk_d), ("wv", wv_d), ("wo", wo_d)):
                t_ = wat_p.tile([128, KDH, 2, D], FP8, name=f"l{l}_{nm}", tag=nm)
                nc.sync.dma_start(t_, d_[l])
                w[nm] = t_
            return w

        def load_kv0(g):
            t_ = kv0_p.tile([128, KDH, 2, GC0], FP8, name=f"kv0g{g}", tag="kv0")
            nc.sync.dma_start(t_, kv0_d[g])
            return t_

        wts = [load_attn_weights(0)]
        kv0g0 = load_kv0(0)

        def fm_transpose(src4, dst, evict, tagname):
            for k in range(KD):
                ps = tp.tile([128, 512], BF16, name=f"{tagname}{k}", tag="tp")
                for i in range(4):
                    nc.tensor.transpose(ps[:, i * 128 : (i + 1) * 128],
                                        src4[i][:, k * 128 : (k + 1) * 128], ident)
                evict(k, ps)

        def emit_attention(l, g, lw, kvg, bqt, bo_t):
            """Attention + LN2 for one example group / token half.
            Returns the h2fm tile feeding this half's FFN."""
            xg = xt[g * GE : (g + 1) * GE]
            h1tm = _layer_norm_half(nc, stats, xg, htm_p, f"l{l}a{g}")
            h1fm = h1fm_p.tile([128, KD, GT], FP8, name=f"l{l}h1fm{g}",
                               tag="h1fm")
            fm_transpose(
                h1tm, h1fm,
                lambda k, ps: nc.scalar.activation(
                    h1fm[:, k, :], ps, ACT.Copy, scale=HSC),
                f"l{l}a{g}tp")

            TCB = C if l == 0 else S
            TCG = GE * TCB

            # ---- Q projection (DoubleRow fp8, raw eviction) ----
            qg = qg_p.tile([128, MD, GT], BF16, name=f"l{l}g{g}_q", tag="qg")
            for m in range(MD):
                ps = big.tile([128, 512], F32, name="qps", tag="big")
                for p in range(KDH):
                    nc.tensor.matmul(
                        ps, lw["wq"][:, p, :, m * 128 : (m + 1) * 128],
                        h1fm[:, 2 * p : 2 * p + 2, :],
                        start=(p == 0), stop=(p == KDH - 1), perf_mode=DR)
                if nonzero_bq:
                    nc.scalar.activation(qg[:, m, :], ps, ACT.Identity,
                                         bias=bqt[:, m : m + 1])
                else:
                    nc.scalar.activation(qg[:, m, :], ps, ACT.Copy)
            # ---- K projection ----
            kg = kg_p.tile([128, MD, GC0], BF16, name=f"l{l}g{g}_k", tag="kg")
            for m in range(MD):
                for n0 in range(0, TCG, 512):
                    n1 = min(n0 + 512, TCG)
                    ps = big.tile([128, 512], F32, name="kps", tag="big")
                    for p in range(KDH):
                        rhs = (kvg[:, p, :, n0:n1] if l == 0
                               else h1fm[:, 2 * p : 2 * p + 2, n0:n1])
                        nc.tensor.matmul(
                            ps[:, : n1 - n0],
                            lw["wk"][:, p, :, m * 128 : (m + 1) * 128],
                            rhs, start=(p == 0), stop=(p == KDH - 1),
                            perf_mode=DR)
                    nc.scalar.activation(kg[:, m, n0:n1], ps[:, : n1 - n0],
                                         ACT.Copy)
            # ---- V projection (token-major, per example, raw) ----
            vts = []
            for e in range(GE):
                segs = []
                for s0 in range(0, TCB, 128):
                    nr = min(128, TCB - s0)
                    vt = vt_p.tile([128, D], BF16, name=f"l{l}g{g}e{e}v{s0}",
                                   tag="v")
                    for n in range(2):
                        ps = big.tile([128, 512], F32, name="vps", tag="big")
                        for p in range(KDH):
                            if l == 0:
                                lh = kvg[:, p, :,
                                         e * TCB + s0 : e * TCB + s0 + nr]
                            else:
                                lh = h1fm[:, 2 * p : 2 * p + 2,
                                          e * 128 : e * 128 + 128]
                            nc.tensor.matmul(
                                ps[:nr], lh,
                                lw["wv"][:, p, :, n * 512 : (n + 1) * 512],
                                start=(p == 0), stop=(p == KDH - 1),
                                perf_mode=DR)
                        nc.scalar.activation(
                            vt[:nr, n * 512 : (n + 1) * 512], ps[:nr],
                            ACT.Copy)
                    segs.append((vt, nr))
                vts.append(segs)

            # ---- attention smalls ----
            ag = ag_p.tile([128, MD, GT], FP8, name=f"l{l}g{g}_a", tag="ag")
            bh = 4 if l > 0 else 2
            nseg = (TCB + 127) // 128
            for e in range(GE):
                for hb in range(0, H, bh):
                    praw = praw_p.tile([128, bh, TCB], BF16, name="praw",
                                       tag="praw")
                    zrow = stats.tile([128, bh], F32, name="zrow", tag="zrow")
                    for hi in range(bh):
                        h_ = hb + hi
                        po, ch = 64 * (h_ % 2), h_ // 2
                        sp = sa.tile([128, TCB], F32, name="sp", tag="sa")
                        nc.tensor.matmul(
                            sp,
                            qg[po : po + 64, ch, e * 128 : (e + 1) * 128],
                            kg[po : po + 64, ch, e * TCB : (e + 1) * TCB],
                            start=True, stop=True)
                        nc.scalar.activation(praw[:, hi, :], sp, ACT.Exp,
                                             scale=ESK[l],
                                             accum_out=zrow[:, hi : hi + 1])
                    rinv = stats.tile([128, bh], F32, name="rinv", tag="rinv")
                    nc.vector.reciprocal(rinv, zrow)
                    pbf = praw
                    nc.vector.tensor_tensor(
                        pbf, praw,
                        rinv[:, :, None].broadcast_to((128, bh, TCB)),
                        OP.mult)
                    if l > 0:
                        col = lambda i: (i % 2) * 2 + i // 2
                        tpp = tp.tile([128, bh, 128], BF16, name="ptp",
                                      tag="tp")
                        for hi in range(bh):
                            nc.tensor.transpose(tpp[:, hi, :],
                                                pbf[:, hi, :], ident)
                        pts = pts_p.tile([128, bh, 128], BF16, name="pts",
                                         tag="pts")
                        nc.vector.tensor_copy(pts, tpp)
                        aps = sa.tile([64, bh, 128], F32, name="aps", tag="sa")
                        for hi in range(bh):
                            h_ = hb + hi
                            vt, _ = vts[e][0]
                            nc.tensor.matmul(
                                aps[:, col(hi), :],
                                vt[:, h_ * 64 : (h_ + 1) * 64],
                                pts[:, hi, :], start=True, stop=True)
                        ch0 = hb // 2
                        for half in range(2):
                            nc.vector.tensor_scalar(
                                ag[half * 64 : half * 64 + 64,
                                   ch0 : ch0 + 2,
                                   e * 128 : (e + 1) * 128],
                                aps[:, half * 2 : half * 2 + 2, :],
                                HSC * SV[l], None, OP.mult)
                    else:
                        tpp = tp.tile([128, nseg, bh, 128], BF16,
                                      name="ptp0", tag="tp")
                        for hi in range(bh):
                            for si in range(nseg):
                                nr = min(128, TCB - si * 128)
                                nc.tensor.transpose(
                                    tpp[:nr, si, hi, :],
                                    pbf[:, hi, si * 128 : si * 128 + nr],
                                    ident)
                        pts = pts_p.tile([128, nseg, bh, 128], BF16,
                                         name="pts0", tag="pts")
                        for si in range(nseg):
                            nr = min(128, TCB - si * 128)
                            nc.vector.tensor_copy(pts[:nr, si], tpp[:nr, si])
                        for hi in range(bh):
                            h_ = hb + hi
                            po, ch = 64 * (h_ % 2), h_ // 2
                            aps = sa.tile([64, 128], F32, name="aps0", tag="sa")
                            for si in range(nseg):
                                nr = min(128, TCB - si * 128)
                                vt, _ = vts[e][si]
                                nc.tensor.matmul(
                                    aps, vt[:nr, h_ * 64 : (h_ + 1) * 64],
                                    pts[:nr, si, hi, :],
                                    start=(si == 0), stop=(si == nseg - 1))
                            nc.vector.tensor_scalar(
                                ag[po : po + 64, ch, e * 128 : (e + 1) * 128],
                                aps, HSC * SV[l], None, OP.mult)

            # ---- output projection, residual add ----
            for e in range(GE):
                xi = xt[g * GE + e]
                for n in range(2):
                    ps = big.tile([128, 512], F32, name="ops", tag="big")
                    for p in range(KDH):
                        nc.tensor.matmul(
                            ps, ag[:, 2 * p : 2 * p + 2,
                                   e * 128 : (e + 1) * 128],
                            lw["wo"][:, p, :, n * 512 : (n + 1) * 512],
                            start=(p == 0), stop=(p == KDH - 1),
                            perf_mode=DR)
                    ot = otmp_p.tile([128, 512], BF16, name="ot", tag="ot")
                    nc.scalar.activation(ot, ps, ACT.Copy, scale=SO[l])
                    nc.vector.tensor_tensor(
                        xi[:, n * 512 : (n + 1) * 512],
                        xi[:, n * 512 : (n + 1) * 512], ot, OP.add)
            if nonzero_bo:
                for i in range(GE):
                    nc.vector.tensor_tensor(xt[g * GE + i], xt[g * GE + i],
                                            bo_t, OP.add)

            # ---- LN2 + transposes for this half ----
            h2tm = _layer_norm_half(nc, stats, xg, htm_p, f"l{l}f{g}")
            h2fm = h2fm_p.tile([128, KD, GT], BF16, name=f"l{l}h2fm{g}",
                               tag="h2fm")
            fm_transpose(
                h2tm, h2fm,
                lambda k, ps: nc.scalar.activation(h2fm[:, k, :], ps, ACT.Copy),
                f"l{l}f{g}tp")
            return h2fm

        def emit_ffn(l, g, h2fm, b1t, b2t):
            """FFN for one token half (512 tokens); dense PE filler block."""
            for fbp in range(NFB // 2):
                fbs = (2 * fbp, 2 * fbp + 1)
                w1t, w2t = {}, {}
                for fb in fbs:
                    w1t[fb] = wffn_p.tile([128, FBT * KD * 128], BF16,
                                          name=f"l{l}g{g}w1_{fb}", tag="w1")
                    nc.gpsimd.dma_start(w1t[fb], w1_d[l, fb])
                    w2t[fb] = wffn_p.tile([128, FBT, D], BF16,
                                          name=f"l{l}g{g}w2_{fb}", tag="w2")
                    nc.gpsimd.dma_start(w2t[fb], w2_d[l, fb])
                uts = []
                for fb in fbs:
                    for ft_ in range(FBT):
                        ps = fps.tile([128, 512], F32, name="ups", tag="fps")
                        for k in range(KD):
                            o0 = ft_ * KD * 128 + k * 128
                            nc.tensor.matmul(
                                ps, w1t[fb][:, o0 : o0 + 128], h2fm[:, k, :],
                                start=(k == 0), stop=(k == KD - 1))
                        ut = u_p.tile([128, 512], BF16, name=f"u{fb}_{ft_}",
                                      tag="u")
                        if nonzero_b1:
                            fcol = fb * FBT + ft_
                            nc.scalar.activation(
                                ut, ps, ACT.Gelu_apprx_tanh,
                                bias=b1t[:, fcol : fcol + 1])
                        else:
                            nc.scalar.activation(ut, ps, ACT.Gelu_apprx_tanh)
                        uts.append(ut)
                for m in range(GE):
                    xi = xt[g * GE + m]
                    for n in range(2):
                        yp = fps.tile([128, 512], F32, name="yp", tag="fps")
                        idx = 0
                        for fi, fb in enumerate(fbs):
                            for kf in range(FBT):
                                nc.tensor.matmul(
                                    yp,
                                    uts[fi * FBT + kf][:, m * 128 :
                                                       (m + 1) * 128],
                                    w2t[fb][:, kf, n * 512 : (n + 1) * 512],
                                    start=(idx == 0), stop=(idx == 2 * FBT - 1))
                                idx += 1
                        nc.vector.tensor_tensor(
                            xi[:, n * 512 : (n + 1) * 512],
                            xi[:, n * 512 : (n + 1) * 512], yp, OP.add)
            if nonzero_b2:
                for i in range(GE):
                    nc.vector.tensor_tensor(xt[g * GE + i], xt[g * GE + i],
                                            b2t, OP.add)

        # ---- half-layer software pipeline: A(l,g) then F(previous half) ----
        pend = None
        for l in range(n_layers):
            bqt = b1t = bo_t = b2t = None
            if nonzero_bq:
                bqt = bias_p.tile([128, MD], F32, name=f"l{l}_bq", tag="bq")
                nc.sync.dma_start(bqt, bq_d[l])
            if nonzero_b1:
                b1t = bias_p.tile([128, FT], F32, name=f"l{l}_b1", tag="b1")
                nc.sync.dma_start(b1t, b1_d[l])
            if nonzero_bo:
                bo_t = bias_p.tile([128, D], F32, name=f"l{l}_bo", tag="bo")
                nc.sync.dma_start(bo_t, bo_d[l])
            if nonzero_b2:
                b2t = bias_p.tile([128, D], F32, name=f"l{l}_b2", tag="b2")
                nc.sync.dma_start(b2t, b2_d[l])
            for g in range(NGRP):
                kvg = (kv0g0 if g == 0 else load_kv0(1)) if l == 0 else None
                h2 = emit_attention(l, g, wts[0], kvg, bqt, bo_t)
                if g == 1 and l + 1 < n_layers:
                    wts[0] = load_attn_weights(l + 1)
                if pend is not None:
                    emit_ffn(*pend)
                pend = (l, g, h2, b1t, b2t)
        emit_ffn(*pend)

        for i in range(BL):
            nc.sync.dma_start(y_out[i * 128 : (i + 1) * 128, :], xt[i])

    _split_multi_waits(nc)
    return nc


def _pair_layout(w):
    """[D_in, N] -> [128, KDH, 2, N] pair layout for DoubleRow."""
    n = w.shape[1]
    return np.ascontiguousarray(
        w.reshape(KDH, 2, 128, n).transpose(2, 0, 1, 3))


def _fp8_scale(w):
    m = float(np.abs(w).max())
    if m == 0.0:
        return 1.0
    return float(2.0 ** np.floor(np.log2(192.0 / m)))


def prepare_host(inputs, n_layers=L):
    """Fold LN affines + biases into weights; fp8-quantize QKVO; arrange
    DMA-friendly layouts."""
    f32 = np.float32
    Wq = np.asarray(inputs["Wq"], f32)
    Wk = np.asarray(inputs["Wk"], f32)
    Wv = np.asarray(inputs["Wv"], f32)
    Wo = np.asarray(inputs["Wo"], f32)
    W1 = np.asarray(inputs["W1"], f32)
    W2 = np.asarray(inputs["W2"], f32)
    bq = np.asarray(inputs["bq"], f32)
    bv = np.asarray(inputs["bv"], f32)
    bo = np.asarray(inputs["bo"], f32)
    b1 = np.asarray(inputs["b1"], f32)
    b2 = np.asarray(inputs["b2"], f32)
    g1 = np.asarray(inputs["ln1_g"], f32)
    be1 = np.asarray(inputs["ln1_b"], f32)
    g2 = np.asarray(inputs["ln2_g"], f32)
    be2 = np.asarray(inputs["ln2_b"], f32)

    scale = np.float32(1.0 / np.sqrt(DK))
    Wq_e = (g1[:, :, None] * Wq) * scale
    bq_e = (bq + np.einsum("ld,ldo->lo", be1, Wq)) * scale
    Wk_e = Wk.copy()
    Wv_e = Wv.copy()
    bv_e = bv.copy()
    for l in range(1, L):
        Wk_e[l] = g1[l][:, None] * Wk[l]
        Wv_e[l] = g1[l][:, None] * Wv[l]
        bv_e[l] = bv[l] + be1[l] @ Wv[l]
    bo_e = bo + np.einsum("ld,ldo->lo", bv_e, Wo)
    W1_e = g2[:, :, None] * W1
    b1_e = b1 + np.einsum("ld,ldo->lo", be2, W1)

    wq8 = np.empty((L, 128, KDH, 2, D), NP_FP8)
    wk8 = np.empty((L, 128, KDH, 2, D), NP_FP8)
    wv8 = np.empty((L, 128, KDH, 2, D), NP_FP8)
    wo8 = np.empty((L, 128, KDH, 2, D), NP_FP8)
    SV, SO, ESK = [], [], []
    for l in range(L):
        sq = _fp8_scale(Wq_e[l])
        wq8[l] = _pair_layout(Wq_e[l] * sq).astype(NP_FP8)
        sk = _fp8_scale(Wk_e[l])
        wk8[l] = _pair_layout(Wk_e[l] * sk).astype(NP_FP8)
        sv = _fp8_scale(Wv_e[l])
        wv8[l] = _pair_layout(Wv_e[l] * sv).astype(NP_FP8)
        so = _fp8_scale(Wo[l])
        wo8[l] = _pair_layout(Wo[l] * so).astype(NP_FP8)
        # scores psum = (16 sq qhat)·(16 sk k) -> exp scale
        ESK.append(float(1.0 / (HSC * HSC * sq * sk)))
        # vt holds 16·sv·v; ag = aps * HSC*SV = HSC·a
        SV.append(float(1.0 / (HSC * sv)))
        # O psum = (HSC a)·(so Wo) -> ot scale
        SO.append(float(1.0 / (HSC * so)))

    w1h = np.empty((L, NFB, 128, FBT * KD * 128), NP_BF16)
    w2h = np.empty((L, NFB, 128, FBT, D), NP_BF16)
    for l in range(L):
        a = W1_e[l].reshape(KD, 128, FT, 128).transpose(1, 2, 0, 3)  # r,ft,k,c
        w1h[l] = (a.reshape(128, NFB, FBT, KD * 128).transpose(1, 0, 2, 3)
                  .reshape(NFB, 128, FBT * KD * 128).astype(NP_BF16))
        b_ = W2[l].reshape(NFB, FBT, 128, D).transpose(0, 2, 1, 3)  # fb,r,ft,o
        w2h[l] = b_.astype(NP_BF16)

    meta = {
        "SV": SV, "SO": SO, "ESK": ESK,
        "nonzero_bq": bool(np.any(bq_e)),
        "nonzero_b1": bool(np.any(b1_e)),
        "nonzero_bo": bool(np.any(bo_e)),
        "nonzero_b2": bool(np.any(b2)),
    }

    host = {"wq": wq8, "wk": wk8, "wv": wv8, "wo": wo8, "w1": w1h, "w2": w2h}
    if meta["nonzero_bq"]:
        host["bq"] = np.ascontiguousarray(
            bq_e.reshape(L, MD, 128).transpose(0, 2, 1))
    if meta["nonzero_b1"]:
        host["b1"] = np.ascontiguousarray(
            b1_e.reshape(L, FT, 128).transpose(0, 2, 1))
    if meta["nonzero_bo"]:
        host["bo_bc"] = np.ascontiguousarray(
            np.broadcast_to(bo_e[:, None, :], (L, 128, D)).astype(f32))
    if meta["nonzero_b2"]:
        host["b2_bc"] = np.ascontiguousarray(
            np.broadcast_to(b2[:, None, :], (L, 128, D)).astype(f32))

    xt = np.asarray(inputs["xt"], f32)
    p_att = np.asarray(inputs["p_att_feats"], f32)
    per_core = []
    for c in range(NCORES):
        xs = np.ascontiguousarray(xt[c * BL : (c + 1) * BL].reshape(T, D))
        kv = np.empty((NGRP, 128, KDH, 2, GC0), NP_FP8)
        for g in range(NGRP):
            blk = p_att[c * BL + g * GE : c * BL + (g + 1) * GE]  # [GE, C, D]
            ft = blk.reshape(GC0, D).T  # [D, GC0]
            kv[g] = _pair_layout(ft * HSC).astype(NP_FP8)
        m = dict(host)
        m["x"] = xs
        m["kv0"] = kv
        per_core.append(m)
    return per_core, meta


def run(inputs, n_layers=L):
    per_core, meta = prepare_host(inputs, n_layers)
    nc = build_program(meta, n_layers)
    res = run_bass_kernel_spmd(nc, per_core, list(range(NCORES)))
    out = np.empty((B, S, D), np.float32)
    for c in range(NCORES):
        out[c * BL : (c + 1) * BL] = res.results[c]["y"].reshape(BL, S, D)
    return out


def kernel(**inputs) -> np.ndarray:
    return run(inputs)


# revision 22
# speedup vs baseline: 1.2195x; 1.0000x over previous
"""Trainium2 Bass kernel for nn_BertAoA_Decoder_Core (6-layer BERT-style encoder,
layer-0 cross-attention to p_att_feats).

Strategy: pure data-parallel over batch across 8 NeuronCores (8 examples/core).
v4: fp8e4m3 DoubleRow matmuls (2x PE rate) for the Q/K/V/O projections with
power-of-2 weight/activation scaling; all projection descales folded into the
softmax Exp scale / ag eviction, so Q/K/V PSUM evictions are plain copies;
softmax denominator via the Exp activation's accum_out (no tensor_reduce);
bf16 FFN weights/activations, fp32 residual stream; FFN weights loaded once
per layer (token-half inner loop); a dedicated PSUM tag for the O-projection
so it never blocks FFN1 slots; weight DMAs on the gpsimd queue.
"""

import sys

sys.path.insert(0, "/opt/trn_rl_repo")

import numpy as np
import ml_dtypes
from contextlib import ExitStack

import concourse.bass as bass
import concourse.mybir as mybir
import concourse.tile as tile
from concourse.masks import make_identity
from concourse.bass_utils import run_bass_kernel_spmd

F32 = mybir.dt.float32
BF16 = mybir.dt.bfloat16
FP8 = mybir.dt.float8e4
AX = mybir.AxisListType.X
OP = mybir.AluOpType
ACT = mybir.ActivationFunctionType
DR = mybir.MatmulPerfMode.DoubleRow

NP_BF16 = ml_dtypes.bfloat16
NP_FP8 = ml_dtypes.float8_e4m3

# Problem constants (hardcoded per contract)
B, S, C, D, H, L, F = 64, 128, 196, 1024, 16, 6, 4096
DK = D // H              # 64
NCORES = 8
BL = B // NCORES         # 8 examples per core
T = BL * S               # 1024 query tokens per core
KD = D // 128            # 8 contraction tiles
KDH = KD // 2            # 4 contraction pair-tiles (DoubleRow)
MD = D // 128            # 8 output tiles
FT = F // 128            # 32 FFN f-tiles
NFB = 8                  # FFN f-blocks
FBT = FT // NFB          # 4 f-tiles per block
NGRP = 2                 # example groups per core (== token halves)
GE = BL // NGRP          # 4 examples per group
GT = GE * S              # 512 tokens per group
GC0 = GE * C             # 784 context tokens per group (layer 0)
LN_EPS = 1e-6
HSC = 16.0               # fp8 activation scale


def _split_multi_waits(nc):
    """This container's walrus accepts only one sync-wait per CTRL instruction;
    hoist extra waits onto preceding NoOps on the same engine."""
    cnt = 0
    for fn in nc.m.functions:
        for bb in fn.blocks:
            new_list = []
            for ins in bb.instructions:
                si = getattr(ins, "sync_info", None)
                ow = getattr(si, "on_wait", None) if si is not None else None
                if ow and len(ow) > 1:
                    for w in ow[:-1]:
                        nop = mybir.InstNoOp(
                            name=f"{ins.name}-wsplit-{cnt}",
                            engine=ins.engine,
                            sync_info=mybir.SyncInfo(on_wait=[w], on_update=[]),
                        )
                        cnt += 1
                        new_list.append(nop)
                    si.on_wait = [ow[-1]]
                new_list.append(ins)
            bb.instructions = new_list
    return cnt


def _newton_rsqrt(nc, pool, v_ap, out_ap, n):
    """out = 1/sqrt(v) elementwise on a small [128, n] fp32 AP, DVE-only."""
    r = pool.tile([128, n], F32, name="rs_r", tag="rs_r")
    t = pool.tile([128, n], F32, name="rs_t", tag="rs_t")
    nc.vector.reciprocal(r, v_ap)
    nc.vector.tensor_scalar(out_ap, r, 0.5, 0.5, OP.mult, OP.add)
    for _ in range(4):
        nc.vector.tensor_tensor(t, out_ap, out_ap, OP.mult)
        nc.vector.tensor_tensor(t, t, v_ap, OP.mult)
        nc.vector.tensor_scalar(t, t, -0.5, 1.5, OP.mult, OP.add)
        nc.vector.tensor_tensor(out_ap, out_ap, t, OP.mult)


def _layer_norm_half(nc, stats_pool, x_tiles, h_pool, tagpfx):
    """Pre-norm (x-mu)*rstd for 4 [128, D] fp32 token-major tiles -> bf16."""
    nb = len(x_tiles)
    stat = stats_pool.tile([128, nb, 12], F32, name=f"{tagpfx}_stat",
                           tag="ln_stat")
    mv = stats_pool.tile([128, nb, 2], F32, name=f"{tagpfx}_mv", tag="ln_mv")
    var = stats_pool.tile([128, nb], F32, name=f"{tagpfx}_var", tag="ln_var")
    rst = stats_pool.tile([128, nb], F32, name=f"{tagpfx}_rst", tag="ln_rst")
    for i in range(nb):
        nc.vector.bn_stats(stat[:, i, 0:6], x_tiles[i][:, 0:512])
        nc.vector.bn_stats(stat[:, i, 6:12], x_tiles[i][:, 512:1024])
        nc.vector.bn_aggr(mv[:, i, :], stat[:, i, :])
    nc.vector.tensor_scalar(var, mv[:, :, 1], LN_EPS, None, OP.add)
    _newton_rsqrt(nc, stats_pool, var, rst, nb)
    h_tiles = []
    for i in range(nb):
        h = h_pool.tile([128, D], BF16, name=f"{tagpfx}_h{i}", tag="htm")
        nc.vector.tensor_scalar(h, x_tiles[i], mv[:, i, 0:1],
                                rst[:, i : i + 1], OP.subtract, OP.mult)
        h_tiles.append(h)
    return h_tiles


def build_program(meta, n_layers=L):
    """meta: dict from prepare_host with descale constants and bias flags."""
    SV, SO, ESK = meta["SV"], meta["SO"], meta["ESK"]
    nonzero_bq, nonzero_b1 = meta["nonzero_bq"], meta["nonzero_b1"]
    nonzero_bo, nonzero_b2 = meta["nonzero_bo"], meta["nonzero_b2"]

    nc = bass.Bass()
    x_in = nc.declare_dram_parameter("x", [T, D], F32, isOutput=False)
    y_out = nc.declare_dram_parameter("y", [T, D], F32, isOutput=True)
    kv0_d = nc.declare_dram_parameter("kv0", [NGRP, 128, KDH, 2, GC0], FP8,
                                      isOutput=False)
    wq_d = nc.declare_dram_parameter("wq", [L, 128, KDH, 2, D], FP8, isOutput=False)
    wk_d = nc.declare_dram_parameter("wk", [L, 128, KDH, 2, D], FP8, isOutput=False)
    wv_d = nc.declare_dram_parameter("wv", [L, 128, KDH, 2, D], FP8, isOutput=False)
    wo_d = nc.declare_dram_parameter("wo", [L, 128, KDH, 2, D], FP8, isOutput=False)
    w1_d = nc.declare_dram_parameter("w1", [L, NFB, 128, FBT * KD * 128], BF16,
                                     isOutput=False)
    w2_d = nc.declare_dram_parameter("w2", [L, NFB, 128, FBT, D], BF16,
                                     isOutput=False)
    if nonzero_bq:
        bq_d = nc.declare_dram_parameter("bq", [L, 128, MD], F32, isOutput=False)
    if nonzero_b1:
        b1_d = nc.declare_dram_parameter("b1", [L, 128, FT], F32, isOutput=False)
    if nonzero_bo:
        bo_d = nc.declare_dram_parameter("bo_bc", [L, 128, D], F32, isOutput=False)
    if nonzero_b2:
        b2_d = nc.declare_dram_parameter("b2_bc", [L, 128, D], F32, isOutput=False)

    with tile.TileContext(nc) as tc, ExitStack() as top:
        const = top.enter_context(tc.tile_pool(name="const", bufs=1))
        ident = const.tile([128, 128], BF16, name="ident_bf")
        make_identity(nc, ident)

        xpool = top.enter_context(tc.tile_pool(name="xres", bufs=BL))
        stats = top.enter_context(tc.tile_pool(name="stats", bufs=2))
        htm_p = top.enter_context(tc.tile_pool(name="htm", bufs=4))
        h1fm_p = top.enter_context(tc.tile_pool(name="h1fm", bufs=2))
        h2fm_p = top.enter_context(tc.tile_pool(name="h2fm", bufs=2))
        wat_p = top.enter_context(tc.tile_pool(name="wat", bufs=1))
        wffn_p = top.enter_context(tc.tile_pool(name="wffn", bufs=2))
        kv0_p = top.enter_context(tc.tile_pool(name="kv0", bufs=2))
        qg_p = top.enter_context(tc.tile_pool(name="qg", bufs=2))
        kg_p = top.enter_context(tc.tile_pool(name="kg", bufs=2))
        vt_p = top.enter_context(tc.tile_pool(name="vt", bufs=5))
        ag_p = top.enter_context(tc.tile_pool(name="ag", bufs=1))
        praw_p = top.enter_context(tc.tile_pool(name="praw", bufs=2))
        pts_p = top.enter_context(tc.tile_pool(name="pts", bufs=1))
        u_p = top.enter_context(tc.tile_pool(name="u", bufs=8))
        otmp_p = top.enter_context(tc.tile_pool(name="otmp", bufs=1))
        bias_p = top.enter_context(tc.tile_pool(name="bias", bufs=2))
        big = top.enter_context(tc.tile_pool(name="big", bufs=2, space="PSUM"))
        fps = top.enter_context(tc.tile_pool(name="fps", bufs=3, space="PSUM"))
        sa = top.enter_context(tc.tile_pool(name="sa", bufs=2, space="PSUM"))
        tp = top.enter_context(tc.tile_pool(name="tp", bufs=1, space="PSUM"))

        xt = []
        for i in range(BL):
            t_ = xpool.tile([128, D], F32, name=f"x{i}", tag="x")
            nc.sync.dma_start(t_, x_in[i * 128 : (i + 1) * 128, :])
            xt.append(t_)

        def load_attn_weights(l):
            w = {}
            for nm, d_ in (("wq", wq_d), ("wk", wk_d), ("wv", wv_d), ("wo", wo_d)):
                t_ = wat_p.tile([128, KDH, 2, D], FP8, name=f"l{l}_{nm}", tag=nm)
                nc.scalar.dma_start(t_, d_[l])
                w[nm] = t_
            return w

        def load_kv0(g):
            t_ = kv0_p.tile([128, KDH, 2, GC0], FP8, name=f"kv0g{g}", tag="kv0")
            nc.scalar.dma_start(t_, kv0_d[g])
            return t_

        wts = [load_attn_weights(0)]
        kv0g0 = load_kv0(0)

        def fm_transpose(src4, dst, evict, tagname):
            for k in range(KD):
                ps = tp.tile([128, 512], BF16, name=f"{tagname}{k}", tag="tp")
                for i in range(4):
                    nc.tensor.transpose(ps[:, i * 128 : (i + 1) * 128],
                                        src4[i][:, k * 128 : (k + 1) * 128], ident)
                evict(k, ps)

        def emit_attention(l, g, lw, kvg, bqt, bo_t):
            """Attention + LN2 for one example group / token half.
            Returns the h2fm tile feeding this half's FFN."""
            xg = xt[g * GE : (g + 1) * GE]
            h1tm = _layer_norm_half(nc, stats, xg, htm_p, f"l{l}a{g}")
            h1fm = h1fm_p.tile([128, KD, GT], FP8, name=f"l{l}h1fm{g}",
                               tag="h1fm")
            fm_transpose(
                h1tm, h1fm,
                lambda k, ps: nc.vector.tensor_scalar(
                    h1fm[:, k, :], ps, HSC, None, OP.mult),
                f"l{l}a{g}tp")

            TCB = C if l == 0 else S
            TCG = GE * TCB

            # ---- Q projection (DoubleRow fp8, raw eviction) ----
            qg = qg_p.tile([128, MD, GT], BF16, name=f"l{l}g{g}_q", tag="qg")
            for m in range(MD):
                ps = big.tile([128, 512], F32, name="qps", tag="big")
                for p in range(KDH):
                    nc.tensor.matmul(
                        ps, lw["wq"][:, p, :, m * 128 : (m + 1) * 128],
                        h1fm[:, 2 * p : 2 * p + 2, :],
                        start=(p == 0), stop=(p == KDH - 1), perf_mode=DR)
                if nonzero_bq:
                    nc.scalar.activation(qg[:, m, :], ps, ACT.Identity,
                                         bias=bqt[:, m : m + 1])
                else:
                    nc.scalar.activation(qg[:, m, :], ps, ACT.Copy)
            # ---- K projection ----
            kg = kg_p.tile([128, MD, GC0], BF16, name=f"l{l}g{g}_k", tag="kg")
            for m in range(MD):
                for n0 in range(0, TCG, 512):
                    n1 = min(n0 + 512, TCG)
                    ps = big.tile([128, 512], F32, name="kps", tag="big")
                    for p in range(KDH):
                        rhs = (kvg[:, p, :, n0:n1] if l == 0
                               else h1fm[:, 2 * p : 2 * p + 2, n0:n1])
                        nc.tensor.matmul(
                            ps[:, : n1 - n0],
                            lw["wk"][:, p, :, m * 128 : (m + 1) * 128],
                            rhs, start=(p == 0), stop=(p == KDH - 1),
                            perf_mode=DR)
                    nc.scalar.activation(kg[:, m, n0:n1], ps[:, : n1 - n0],
                                         ACT.Copy)
            # ---- V projection (token-major, per example, raw) ----
            vts = []
            for e in range(GE):
                segs = []
                for s0 in range(0, TCB, 128):
                    nr = min(128, TCB - s0)
                    vt = vt_p.tile([128, D], BF16, name=f"l{l}g{g}e{e}v{s0}",
                                   tag="v")
                    for n in range(2):
                        ps = big.tile([128, 512], F32, name="vps", tag="big")
                        for p in range(KDH):
                            if l == 0:
                                lh = kvg[:, p, :,
                                         e * TCB + s0 : e * TCB + s0 + nr]
                            else:
                                lh = h1fm[:, 2 * p : 2 * p + 2,
                                          e * 128 : e * 128 + 128]
                            nc.tensor.matmul(
                                ps[:nr], lh,
                                lw["wv"][:, p, :, n * 512 : (n + 1) * 512],
                                start=(p == 0), stop=(p == KDH - 1),
                                perf_mode=DR)
                        nc.scalar.activation(
                            vt[:nr, n * 512 : (n + 1) * 512], ps[:nr],
                            ACT.Copy)
                    segs.append((vt, nr))
                vts.append(segs)

            # ---- attention smalls ----
            ag = ag_p.tile([128, MD, GT], FP8, name=f"l{l}g{g}_a", tag="ag")
            bh = 4 if l > 0 else 2
            nseg = (TCB + 127) // 128
            for e in range(GE):
                for hb in range(0, H, bh):
                    praw = praw_p.tile([128, bh, TCB], BF16, name="praw",
                                       tag="praw")
                    zrow = stats.tile([128, bh], F32, name="zrow", tag="zrow")
                    for hi in range(bh):
                        h_ = hb + hi
                        po, ch = 64 * (h_ % 2), h_ // 2
                        sp = sa.tile([128, TCB], F32, name="sp", tag="sa")
                        nc.tensor.matmul(
                            sp,
                            qg[po : po + 64, ch, e * 128 : (e + 1) * 128],
                            kg[po : po + 64, ch, e * TCB : (e + 1) * TCB],
                            start=True, stop=True)
                        nc.scalar.activation(praw[:, hi, :], sp, ACT.Exp,
                                             scale=ESK[l],
                                             accum_out=zrow[:, hi : hi + 1])
                    rinv = stats.tile([128, bh], F32, name="rinv", tag="rinv")
                    nc.vector.reciprocal(rinv, zrow)
                    pbf = praw
                    nc.vector.tensor_tensor(
                        pbf, praw,
                        rinv[:, :, None].broadcast_to((128, bh, TCB)),
                        OP.mult)
                    if l > 0:
                        col = lambda i: (i % 2) * 2 + i // 2
                        tpp = tp.tile([128, bh, 128], BF16, name="ptp",
                                      tag="tp")
                        for hi in range(bh):
                            nc.tensor.transpose(tpp[:, hi, :],
                                                pbf[:, hi, :], ident)
                        pts = pts_p.tile([128, bh, 128], BF16, name="pts",
                                         tag="pts")
                        nc.vector.tensor_copy(pts, tpp)
                        aps = sa.tile([64, bh, 128], F32, name="aps", tag="sa")
                        for hi in range(bh):
                            h_ = hb + hi
                            vt, _ = vts[e][0]
                            nc.tensor.matmul(
                                aps[:, col(hi), :],
                                vt[:, h_ * 64 : (h_ + 1) * 64],
                                pts[:, hi, :], start=True, stop=True)
                        ch0 = hb // 2
                        for half in range(2):
                            nc.vector.tensor_scalar(
                                ag[half * 64 : half * 64 + 64,
                                   ch0 : ch0 + 2,
                                   e * 128 : (e + 1) * 128],
                                aps[:, half * 2 : half * 2 + 2, :],
                                HSC * SV[l], None, OP.mult)
                    else:
                        tpp = tp.tile([128, nseg, bh, 128], BF16,
                                      name="ptp0", tag="tp")
                        for hi in range(bh):
                            for si in range(nseg):
                                nr = min(128, TCB - si * 128)
                                nc.tensor.transpose(
                                    tpp[:nr, si, hi, :],
                                    pbf[:, hi, si * 128 : si * 128 + nr],
                                    ident)
                        pts = pts_p.tile([128, nseg, bh, 128], BF16,
                                         name="pts0", tag="pts")
                        for si in range(nseg):
                            nr = min(128, TCB - si * 128)
                            nc.vector.tensor_copy(pts[:nr, si], tpp[:nr, si])
                        for hi in range(bh):
                            h_ = hb + hi
                            po, ch = 64 * (h_ % 2), h_ // 2
                            aps = sa.tile([64, 128], F32, name="aps0", tag="sa")
                            for si in range(nseg):
                                nr = min(128, TCB - si * 128)
                                vt, _ = vts[e][si]
                                nc.tensor.matmul(
                                    aps, vt[:nr, h_ * 64 : (h_ + 1) * 64],
                                    pts[:nr, si, hi, :],
                                    start=(si == 0), stop=(si == nseg - 1))
                            nc.vector.tensor_scalar(
                                ag[po : po + 64, ch, e * 128 : (e + 1) * 128],
                                aps, HSC * SV[l], None, OP.mult)

            # ---- output projection, residual add ----
            for e in range(GE):
                xi = xt[g * GE + e]
                for n in range(2):
                    ps = big.tile([128, 512], F32, name="ops", tag="big")
                    for p in range(KDH):
                        nc.tensor.matmul(
                            ps, ag[:, 2 * p : 2 * p + 2,
                                   e * 128 : (e + 1) * 128],
                            lw["wo"][:, p, :, n * 512 : (n + 1) * 512],
                            start=(p == 0), stop=(p == KDH - 1),
                            perf_mode=DR)
                    ot = otmp_p.tile([128, 512], BF16, name="ot", tag="ot")
                    nc.scalar.activation(ot, ps, ACT.Copy, scale=SO[l])
                    nc.vector.tensor_tensor(
                        xi[:, n * 512 : (n + 1) * 512],
                        xi[:, n * 512 : (n + 1) * 512], ot, OP.add)
            if nonzero_bo:
                for i in range(GE):
                    nc.vector.tensor_tensor(xt[g * GE + i], xt[g * GE + i],
                                            bo_t, OP.add)

            # ---- LN2 + transposes for this half ----
            h2tm = _layer_norm_half(nc, stats, xg, htm_p, f"l{l}f{g}")
            h2fm = h2fm_p.tile([128, KD, GT], BF16, name=f"l{l}h2fm{g}",
                               tag="h2fm")
            fm_transpose(
                h2tm, h2fm,
                lambda k, ps: nc.vector.tensor_copy(h2fm[:, k, :], ps),
                f"l{l}f{g}tp")
            return h2fm

        def emit_ffn(l, g, h2fm, b1t, b2t):
            """FFN for one token half (512 tokens); dense PE filler block."""
            for fbp in range(NFB // 2):
                fbs = (2 * fbp, 2 * fbp + 1)
                w1t, w2t = {}, {}
                for fb in fbs:
                    w1t[fb] = wffn_p.tile([128, FBT * KD * 128], BF16,
                                          name=f"l{l}g{g}w1_{fb}", tag="w1")
                    nc.gpsimd.dma_start(w1t[fb], w1_d[l, fb])
                    w2t[fb] = wffn_p.tile([128, FBT, D], BF16,
                                          name=f"l{l}g{g}w2_{fb}", tag="w2")
                    nc.gpsimd.dma_start(w2t[fb], w2_d[l, fb])
                uts = []
                for fb in fbs:
                    for ft_ in range(FBT):
                        ps = fps.tile([128, 512], F32, name="ups", tag="fps")
                        for k in range(KD):
                            o0 = ft_ * KD * 128 + k * 128
                            nc.tensor.matmul(
                                ps, w1t[fb][:, o0 : o0 + 128], h2fm[:, k, :],
                                start=(k == 0), stop=(k == KD - 1))
                        ut = u_p.tile([128, 512], BF16, name=f"u{fb}_{ft_}",
                                      tag="u")
                        if nonzero_b1:
                            fcol = fb * FBT + ft_
                            nc.scalar.activation(
                                ut, ps, ACT.Gelu_apprx_tanh,
                                bias=b1t[:, fcol : fcol + 1])
                        else:
                            nc.scalar.activation(ut, ps, ACT.Gelu_apprx_tanh)
                        uts.append(ut)
                for m in range(GE):
                    xi = xt[g * GE + m]
                    for n in range(2):
                        yp = fps.tile([128, 512], F32, name="yp", tag="fps")
                        idx = 0
                        for fi, fb in enumerate(fbs):
                            for kf in range(FBT):
                                nc.tensor.matmul(
                                    yp,
                                    uts[fi * FBT + kf][:, m * 128 :
                                                       (m + 1) * 128],
                                    w2t[fb][:, kf, n * 512 : (n + 1) * 512],
                                    start=(idx == 0), stop=(idx == 2 * FBT - 1))
                                idx += 1
                        nc.vector.tensor_tensor(
                            xi[:, n * 512 : (n + 1) * 512],
                            xi[:, n * 512 : (n + 1) * 512], yp, OP.add)
            if nonzero_b2:
                for i in range(GE):
                    nc.vector.tensor_tensor(xt[g * GE + i], xt[g * GE + i],
                                            b2t, OP.add)

        # ---- half-layer software pipeline: A(l,g) then F(previous half) ----
        pend = None
        for l in range(n_layers):
            bqt = b1t = bo_t = b2t = None
            if nonzero_bq:
                bqt = bias_p.tile([128, MD], F32, name=f"l{l}_bq", tag="bq")
                nc.sync.dma_start(bqt, bq_d[l])
            if nonzero_b1:
                b1t = bias_p.tile([128, FT], F32, name=f"l{l}_b1", tag="b1")
                nc.sync.dma_start(b1t, b1_d[l])
            if nonzero_bo:
                bo_t = bias_p.tile([128, D], F32, name=f"l{l}_bo", tag="bo")
                nc.sync.dma_start(bo_t, bo_d[l])
            if nonzero_b2:
                b2t = bias_p.tile([128, D], F32, name=f"l{l}_b2", tag="b2")
                nc.sync.dma_start(b2t, b2_d[l])
            for g in range(NGRP):
                kvg = (kv0g0 if g == 0 else load_kv0(1)) if l == 0 else None
                h2 = emit_attention(l, g, wts[0], kvg, bqt, bo_t)
                if g == 1 and l + 1 < n_layers:
                    wts[0] = load_attn_weights(l + 1)
                if pend is not None:
                    emit_ffn(*pend)
                pend = (l, g, h2, b1t, b2t)
        emit_ffn(*pend)

        for i in range(BL):
            nc.sync.dma_start(y_out[i * 128 : (i + 1) * 128, :], xt[i])

    _split_multi_waits(nc)
    return nc


def _pair_layout(w):
    """[D_in, N] -> [128, KDH, 2, N] pair layout for DoubleRow."""
    n = w.shape[1]
    return np.ascontiguousarray(
        w.reshape(KDH, 2, 128, n).transpose(2, 0, 1, 3))


def _fp8_scale(w):
    m = float(np.abs(w).max())
    if m == 0.0:
        return 1.0
    return float(2.0 ** np.floor(np.log2(192.0 / m)))


def prepare_host(inputs, n_layers=L):
    """Fold LN affines + biases into weights; fp8-quantize QKVO; arrange
    DMA-friendly layouts."""
    f32 = np.float32
    Wq = np.asarray(inputs["Wq"], f32)
    Wk = np.asarray(inputs["Wk"], f32)
    Wv = np.asarray(inputs["Wv"], f32)
    Wo = np.asarray(inputs["Wo"], f32)
    W1 = np.asarray(inputs["W1"], f32)
    W2 = np.asarray(inputs["W2"], f32)
    bq = np.asarray(inputs["bq"], f32)
    bv = np.asarray(inputs["bv"], f32)
    bo = np.asarray(inputs["bo"], f32)
    b1 = np.asarray(inputs["b1"], f32)
    b2 = np.asarray(inputs["b2"], f32)
    g1 = np.asarray(inputs["ln1_g"], f32)
    be1 = np.asarray(inputs["ln1_b"], f32)
    g2 = np.asarray(inputs["ln2_g"], f32)
    be2 = np.asarray(inputs["ln2_b"], f32)

    scale = np.float32(1.0 / np.sqrt(DK))
    Wq_e = (g1[:, :, None] * Wq) * scale
    bq_e = (bq + np.einsum("ld,ldo->lo", be1, Wq)) * scale
    Wk_e = Wk.copy()
    Wv_e = Wv.copy()
    bv_e = bv.copy()
    for l in range(1, L):
        Wk_e[l] = g1[l][:, None] * Wk[l]
        Wv_e[l] = g1[l][:, None] * Wv[l]
        bv_e[l] = bv[l] + be1[l] @ Wv[l]
    bo_e = bo + np.einsum("ld,ldo->lo", bv_e, Wo)
    W1_e = g2[:, :, None] * W1
    b1_e = b1 + np.einsum("ld,ldo->lo", be2, W1)

    wq8 = np.empty((L, 128, KDH, 2, D), NP_FP8)
    wk8 = np.empty((L, 128, KDH, 2, D), NP_FP8)
    wv8 = np.empty((L, 128, KDH, 2, D), NP_FP8)
    wo8 = np.empty((L, 128, KDH, 2, D), NP_FP8)
    SV, SO, ESK = [], [], []
    for l in range(L):
        sq = _fp8_scale(Wq_e[l])
        wq8[l] = _pair_layout(Wq_e[l] * sq).astype(NP_FP8)
        sk = _fp8_scale(Wk_e[l])
        wk8[l] = _pair_layout(Wk_e[l] * sk).astype(NP_FP8)
        sv = _fp8_scale(Wv_e[l])
        wv8[l] = _pair_layout(Wv_e[l] * sv).astype(NP_FP8)
        so = _fp8_scale(Wo[l])
        wo8[l] = _pair_layout(Wo[l] * so).astype(NP_FP8)
        # scores psum = (16 sq qhat)·(16 sk k) -> exp scale
        ESK.append(float(1.0 / (HSC * HSC * sq * sk)))
        # vt holds 16·sv·v; ag = aps * HSC*SV = HSC·a
        SV.append(float(1.0 / (HSC * sv)))
        # O psum = (HSC a)·(so Wo) -> ot scale
        SO.append(float(1.0 / (HSC * so)))

    w1h = np.empty((L, NFB, 128, FBT * KD * 128), NP_BF16)
    w2h = np.empty((L, NFB, 128, FBT, D), NP_BF16)
    for l in range(L):
        a = W1_e[l].reshape(KD, 128, FT, 128).transpose(1, 2, 0, 3)  # r,ft,k,c
        w1h[l] = (a.reshape(128, NFB, FBT, KD * 128).transpose(1, 0, 2, 3)
                  .reshape(NFB, 128, FBT * KD * 128).astype(NP_BF16))
        b_ = W2[l].reshape(NFB, FBT, 128, D).transpose(0, 2, 1, 3)  # fb,r,ft,o
        w2h[l] = b_.astype(NP_BF16)

    meta = {
        "SV": SV, "SO": SO, "ESK": ESK,
        "nonzero_bq": bool(np.any(bq_e)),
        "nonzero_b1": bool(np.any(b1_e)),
        "nonzero_bo": bool(np.any(bo_e)),
        "nonzero_b2": bool(np.any(b2)),
    }

    host = {"wq": wq8, "wk": wk8, "wv": wv8, "wo": wo8, "w1": w1h, "w2": w2h}
    if meta["nonzero_bq"]:
        host["bq"] = np.ascontiguousarray(
            bq_e.reshape(L, MD, 128).transpose(0, 2, 1))
    if meta["nonzero_b1"]:
        host["b1"] = np.ascontiguousarray(
            b1_e.reshape(L, FT, 128).transpose(0, 2, 1))
    if meta["nonzero_bo"]:
        host["bo_bc"] = np.ascontiguousarray(
            np.broadcast_to(bo_e[:, None, :], (L, 128, D)).astype(f32))
    if meta["nonzero_b2"]:
        host["b2_bc"] = np.ascontiguousarray(
            np.broadcast_to(b2[:, None, :], (L, 128, D)).astype(f32))

    xt = np.asarray(inputs["xt"], f32)
    p_att = np.asarray(inputs["p_att_feats"], f32)
    per_core = []
    for c in range(NCORES):
        xs = np.ascontiguousarray(xt[c * BL : (c + 1) * BL].reshape(T, D))
        kv = np.empty((NGRP, 128, KDH, 2, GC0), NP_FP8)
        for g in range(NGRP):
            blk = p_att[c * BL + g * GE : c * BL + (g + 1) * GE]  # [GE, C, D]
            ft = blk.reshape(GC0, D).T  # [D, GC0]
            kv[g] = _pair_layout(ft * HSC).astype(NP_FP8)
        m = dict(host)
        m["x"] = xs
        m["kv0"] = kv
        per_core.append(m)
    return per_core, meta


def run(inputs, n_layers=L):
    per_core, meta = prepare_host(inputs, n_layers)
    nc = build_program(meta, n_layers)
    res = run_bass_kernel_spmd(nc, per_core, list(range(NCORES)))
    out = np.empty((B, S, D), np.float32)
    for c in range(NCORES):
        out[c * BL : (c + 1) * BL] = res.results[c]["y"].reshape(BL, S, D)
    return out


def kernel(**inputs) -> np.ndarray:
    return run(inputs)
